# revision 11
# baseline (speedup 1.0000x reference)
"""Trainium2 Bass kernel for nn_Attention_65609920414302 (sparse multi-branch attention).

Sharding: 64 total heads (4 branches x 16 sub-heads) split as 8 heads per core
(core c = branch c//2, base-head half c%2). Each core computes Q/K/V projections
for its heads, RoPE, causal thresholded-softplus attention, and a partial W_O
matmul; the host sums the 8 partial outputs.

Math rescaling used on device (S = pi/sqrt(3)):
  reference w_sig = w*sigmoid(S*w) with w = softplus(scores*m), thresholded at sink.
  device   W = silu(S*w) = S*w_sig, thresholded at S*sink,
  probs    = W / (sum_s W + S*(sink+1e-6)),  sink term = S*sink / (...).
The S factors cancel exactly.

Layouts (per core):
  XT   [C=1024, T=1024]   X transposed (f32r)
  Q^T/K^T tiles [128ch, 4, 1024t] (fp16 after RoPE); channel order per head is
      [evens(32) | odds(32)] (host permutes W_Q/W_K columns) so RoPE is a
      32-row swap + cos/sin elementwise.
  V    [128t, 8tt, 8head, 65] (fp16), col 64 = ones (gives sum_s W via PV matmul)
  scores^T tiles [128 s, L] in PSUM per (head, s_tile), L = 1024-128*i (causal)
  Wbuf [128 s, 8 head, 4608] fp16: concatenated valid t-ranges per s_tile
  ctx  [128ch, 4, 1024t] f32r -> W_O (f32r) -> YT [1024 d, 1024 t] partial out
"""

import math
import os
import numpy as np

D_MODEL = 1024
N_HEAD = 16
N_BR = 4
DH = 64
H_TOT = 64
T = 1024
S = math.pi / math.sqrt(3.0)
ATTNSCALE = DH ** -0.5
N_CORES = 8
HPC = 8          # heads per core
KT = 8           # C // 128 contraction tiles
L_LIST = [T - 128 * i for i in range(8)]
O_LIST = [sum(L_LIST[:i]) for i in range(8)]
W_COLS = sum(L_LIST)  # 4608

_NC_CACHE = [None]
LAST_RESULT = [None]  # stash for test harness (exec_time_ns etc.)


def _build_nc():
    import concourse.bass as bass
    from concourse import bacc
    import concourse.mybir as mybir
    import concourse.tile as tile
    from concourse.masks import make_identity

    F32 = mybir.dt.float32
    F32R = mybir.dt.float32r
    F16 = mybir.dt.float16
    AF = mybir.ActivationFunctionType
    ALU = mybir.AluOpType

    nc = bacc.Bacc(None, target_bir_lowering=False, debug=False)

    # ---- DRAM parameters (per-core data; same program on all cores) ----
    XT = nc.declare_dram_parameter("XT", [D_MODEL, T], F16, isOutput=False)
    WQ = nc.declare_dram_parameter("WQ", [D_MODEL, 512], F16, isOutput=False)
    BQ = nc.declare_dram_parameter("BQ", [1, 512], F16, isOutput=False)
    WK = nc.declare_dram_parameter("WK", [D_MODEL, 512], F16, isOutput=False)
    BK = nc.declare_dram_parameter("BK", [1, 512], F16, isOutput=False)
    WV = nc.declare_dram_parameter("WV", [D_MODEL, 512], F16, isOutput=False)
    BV = nc.declare_dram_parameter("BV", [1, 512], F16, isOutput=False)
    WO = nc.declare_dram_parameter("WO", [512, D_MODEL], F32R, isOutput=False)
    COS = nc.declare_dram_parameter("COS", [128, T], F16, isOutput=False)
    SIN = nc.declare_dram_parameter("SIN", [128, T], F16, isOutput=False)
    PSW = nc.declare_dram_parameter("PSW", [128, 128], F16, isOutput=False)
    SEL = nc.declare_dram_parameter("SEL", [128, 4, 8], F16, isOutput=False)
    THR = nc.declare_dram_parameter("THR", [128, 8], F32, isOutput=False)
    TB = nc.declare_dram_parameter("TB", [1, 8], F32, isOutput=False)
    VNS = nc.declare_dram_parameter("VNS", [64, 8], F32, isOutput=False)
    ONES = nc.declare_dram_parameter("ONES", [1, 512], F16, isOutput=False)
    YT = nc.declare_dram_parameter("YT", [D_MODEL, T], F32, isOutput=True)

    with tile.TileContext(nc) as tc:
        pc = tc.alloc_tile_pool(name="const", bufs=1)
        pk = tc.alloc_tile_pool(name="keep", bufs=1)
        tr = tc.alloc_tile_pool(name="trans", bufs=2)
        pp = tc.alloc_tile_pool(name="proj", bufs=1)
        pj = tc.alloc_tile_pool(name="psproj", bufs=1, space="PSUM")

        # ---- constants / weights into SBUF ----
        cos_sb = pc.tile([128, T], F16)
        sin_sb = pc.tile([128, T], F16)
        psw_sb = pc.tile([128, 128], F16)
        sel_sb = pc.tile([128, 4, 8], F16)
        thr_sb = pc.tile([128, 8], F32)
        tb_sb = pc.tile([1, 8], F32)
        vns_sb = pc.tile([64, 8], F32)
        ident = pc.tile([128, 128], F32)
        ones_r = pc.tile([1, 512], F16)
        m_colsb = pc.tile([128, 8, 8], F32)
        m_all = pc.tile([8, T], F32)
        ksc = pc.tile([8, T], F32)

        nc.sync.dma_start(out=cos_sb, in_=COS.ap())
        nc.sync.dma_start(out=sin_sb, in_=SIN.ap())
        nc.sync.dma_start(out=psw_sb, in_=PSW.ap())
        nc.sync.dma_start(out=sel_sb, in_=SEL.ap())
        nc.sync.dma_start(out=thr_sb, in_=THR.ap())
        nc.sync.dma_start(out=tb_sb, in_=TB.ap())
        nc.sync.dma_start(out=vns_sb, in_=VNS.ap())
        make_identity(nc, ident)
        nc.sync.dma_start(out=ones_r, in_=ONES.ap())

        xt = pp.tile([128, KT, T], F16)
        wq = pp.tile([128, KT, 4, 128], F16)
        wk = pp.tile([128, KT, 4, 128], F16)
        wv = pp.tile([128, KT, 512], F16)
        bq = pp.tile([1, 512], F16)
        bk = pp.tile([1, 512], F16)
        bv = pp.tile([1, 512], F16)
        xt_src = XT.ap().rearrange("(kt p) t -> p kt t", p=128)
        wk_src = WK.ap().rearrange("(kt p) (mt m) -> p kt mt m", p=128, m=128)
        for kt in range(KT):
            nc.sync.dma_start(out=xt[:, kt, :], in_=xt_src[:, kt, :])
            nc.sync.dma_start(out=wk[:, kt, :, :], in_=wk_src[:, kt, :, :])
        nc.sync.dma_start(
            out=wq, in_=WQ.ap().rearrange("(kt p) (mt m) -> p kt mt m", p=128, m=128)
        )
        nc.sync.dma_start(out=wv, in_=WV.ap().rearrange("(kt p) v -> p kt v", p=128))
        nc.sync.dma_start(out=bq, in_=BQ.ap())
        nc.sync.dma_start(out=bk, in_=BK.ap())
        nc.sync.dma_start(out=bv, in_=BV.ap())

        wo = pk.tile([128, 4, 8, 128], F32R)
        nc.sync.dma_start(
            out=wo, in_=WO.ap().rearrange("(ct p) (mt m) -> p ct mt m", p=128, m=128)
        )

        qrope = pk.tile([128, 4, T], F16)
        krope = pk.tile([128, 4, T], F16)
        vstore = pk.tile([128, 8, HPC, 65], F16)
        ctx = pk.tile([128, 4, T], F32R)
        nc.vector.memset(vstore[:, :, :, 64:65], 1.0)

        # ---- projection + rope for K and Q ----
        def proj_rope(w_t, b_t, out_t):
            for g in range(4):
                ps = pj.tile([128, T], F32, tag="projps", bufs=2)
                for th in range(2):
                    sl = slice(512 * th, 512 * (th + 1))
                    for kt in range(KT):
                        nc.tensor.matmul(
                            ps[:, sl], w_t[:, kt, g, :], xt[:, kt, sl],
                            start=(kt == 0), stop=False,
                        )
                    nc.tensor.matmul(
                        ps[:, sl], b_t[0:1, 128 * g:128 * (g + 1)], ones_r,
                        start=False, stop=True,
                    )
                qsb = tr.tile([128, T], F16, tag="qsb")
                nc.vector.tensor_copy(qsb, ps)
                sw = pj.tile([128, T], F32, tag="swapps")
                for th in range(2):
                    sl = slice(512 * th, 512 * (th + 1))
                    nc.tensor.matmul(sw[:, sl], psw_sb, qsb[:, sl], start=True, stop=True)
                t1 = tr.tile([128, T], F16, tag="t1")
                nc.vector.tensor_tensor(t1, qsb, cos_sb, op=ALU.mult)
                t2 = tr.tile([128, T], F16, tag="t2")
                nc.vector.tensor_tensor(t2, sw, sin_sb, op=ALU.mult)
                nc.gpsimd.tensor_tensor(out_t[:, g, :], t1, t2, op=ALU.add)

        proj_rope(wk, bk, krope)

        # ---- key_self -> m (scale columns for exp) ----
        ks_ps = pj.tile([8, T], F32, tag="swapps")
        for g in range(4):
            k2 = tr.tile([128, T], F16, tag="k2")
            nc.vector.tensor_tensor(k2, krope[:, g, :], krope[:, g, :], op=ALU.mult)
            for th in range(2):
                sl = slice(512 * th, 512 * (th + 1))
                nc.tensor.matmul(
                    ks_ps[:, sl], sel_sb[:, g, :], k2[:, sl],
                    start=(g == 0), stop=(g == 3),
                )
        nc.vector.tensor_scalar_max(ksc, ks_ps, 1e-6)
        nc.vector.reciprocal_approx_fast(ksc, ksc)
        # m = ATTNSCALE / sqrt(key_self) = sqrt(recip / DH)
        nc.scalar.activation(m_all, ksc, AF.Sqrt, scale=1.0 / DH)
        for i in range(8):
            mt_ps = pj.tile([128, 8], F32, tag="swapps")
            nc.tensor.transpose(mt_ps, m_all[:, 128 * i:128 * (i + 1)], ident[0:8, 0:8])
            nc.vector.tensor_copy(m_colsb[:, i, :], mt_ps)

        proj_rope(wq, bq, qrope)

        # ---- V projection (t on partitions) ----
        for tt_i in range(8):
            psv = pj.tile([128, T], F32, tag="projps", bufs=2)
            for kt in range(KT):
                nc.tensor.matmul(
                    psv[:, 0:512], xt[:, kt, 128 * tt_i:128 * (tt_i + 1)],
                    wv[:, kt, :], start=(kt == 0), stop=False,
                )
            nc.tensor.matmul(
                psv[:, 0:512], ones_r[0:1, 0:128], bv, start=False, stop=True
            )
            nc.vector.tensor_copy(
                vstore[:, tt_i, :, 0:64],
                psv[:, 0:512].rearrange("p (h d) -> p h d", d=64),
            )

        # ---- attention ----
        pp.release()
        pj.release()
        pa = tc.alloc_tile_pool(name="psattn", bufs=1, space="PSUM")
        pw = tc.alloc_tile_pool(name="wbuf", bufs=1)
        wbuf = pw.tile([128, HPC, W_COLS], F16)

        # scores + exp + ln + silu + threshold + PV, in 2 waves of 4 heads.
        # Within a wave: all exps (one table set), then all lns, then silus
        # (dep-enforced ordering keeps ACT table loads to one per phase).
        from concourse.tile import add_dep_helper

        prev_wave_silu = []
        y_acc = pw.tile([128, 8, T], F16)
        for wi, pairs in enumerate([(0, 1, 2), (3,)]):
            exp_insts = []
            for j in pairs:
                for i in range(8):
                    t0 = 128 * i
                    L = L_LIST[i]
                    pss = []
                    for u in range(2):
                        h = 2 * j + u
                        g, r0 = h // 2, 64 * (h % 2)
                        ps_s = pa.tile([128, T], F32, tag="scores", bufs=2)
                        for c0 in range(0, L, 512):
                            c1 = min(c0 + 512, L)
                            nc.tensor.matmul(
                                ps_s[:, c0:c1],
                                krope[r0:r0 + 64, g, t0:t0 + 128],
                                qrope[r0:r0 + 64, g, t0 + c0:t0 + c1],
                                start=True, stop=True,
                            )
                        pss.append((h, ps_s))
                    for h, ps_s in pss:
                        o = O_LIST[i]
                        e = nc.scalar.activation(
                            wbuf[:, h, o:o + L], ps_s[:, 0:L], AF.Exp,
                            scale=m_colsb[:, i, h:h + 1],
                        )
                        for si in prev_wave_silu:
                            add_dep_helper(e.ins, si.ins, sync=False,
                                           reason="act table phase order")
                        exp_insts.append(e)
                        # zero the upper-triangular part of the diagonal block
                        nc.gpsimd.affine_select(
                            out=wbuf[:, h, o:o + 128], in_=wbuf[:, h, o:o + 128],
                            compare_op=ALU.is_ge, fill=0.0, base=0,
                            pattern=[[1, 128]], channel_multiplier=-1,
                        )
            ln_insts = []
            for j in pairs:
                for u in range(2):
                    h = 2 * j + u
                    ln = nc.scalar.activation(
                        wbuf[:, h, :], wbuf[:, h, :], AF.Ln, bias=1.0
                    )
                    for e in exp_insts:
                        add_dep_helper(ln.ins, e.ins, sync=False,
                                       reason="act table phase order")
                    ln_insts.append((h, ln))
            wave_silu = []
            for h, _ln in ln_insts:
                si = nc.scalar.activation(
                    wbuf[:, h, :], wbuf[:, h, :], AF.Silu, scale=S
                )
                for _h2, l2 in ln_insts:
                    add_dep_helper(si.ins, l2.ins, sync=False,
                                   reason="act table phase order")
                wave_silu.append(si)
                # threshold: w = (w >= thr) * w   (mask on 4x path, mult on 2x)
                msk = tr.tile([128, W_COLS], F16, tag="msk", bufs=1)
                nc.vector.tensor_scalar(
                    msk, wbuf[:, h, :], thr_sb[:, h:h + 1], None, op0=ALU.is_ge
                )
                nc.vector.tensor_tensor(
                    wbuf[:, h, :], wbuf[:, h, :], msk, op=ALU.mult
                )
                ps_pv = pa.tile([65, T], F32, tag="pv", bufs=2)
                for i in range(8):
                    t0 = 128 * i
                    o = O_LIST[i]
                    chunks = []
                    if t0 < 512:
                        chunks.append((t0, 512, 3))
                        chunks.append((512, T, 7))
                    else:
                        chunks.append((t0, T, 7))
                    for (a, b, last_i) in chunks:
                        nc.tensor.matmul(
                            ps_pv[:, a:b],
                            vstore[:, i, h, :],
                            wbuf[:, h, o + (a - t0):o + (b - t0)],
                            start=(i == 0), stop=(i == last_i),
                        )
                tp = tr.tile([1, T], F32, tag="tp")
                nc.vector.tensor_scalar_add(tp, ps_pv[64:65, :], tb_sb[0:1, h:h + 1])
                nc.vector.reciprocal_approx_fast(tp, tp)
                gb = tr.tile([64, T], F32, tag="gb")
                nc.gpsimd.partition_broadcast(gb, tp, channels=64)
                r0 = 64 * (h % 2)
                nc.vector.scalar_tensor_tensor(
                    out=ctx[r0:r0 + 64, h // 2, :], in0=ps_pv[0:64, :],
                    scalar=vns_sb[:, h:h + 1], in1=gb,
                    op0=ALU.add, op1=ALU.mult,
                )
            prev_wave_silu = wave_silu

            # W_O partial for this wave's ctx tiles; last wave adds the
            # accumulated first-wave partials and writes out.
            cts = [2 * j_ for j_ in pairs] if False else None
            if wi == 0:
                for mt in range(8):
                    for th in range(2):
                        sl = slice(512 * th, 512 * (th + 1))
                        ps_o = pa.tile([128, T], F32, tag="scores", bufs=2)
                        for ci, ct in enumerate((0, 1, 2)):
                            nc.tensor.matmul(
                                ps_o[:, 0:512], wo[:, ct, mt, :], ctx[:, ct, sl],
                                start=(ci == 0), stop=(ci == 2),
                            )
                        nc.vector.tensor_copy(y_acc[:, mt, sl], ps_o[:, 0:512])
            else:
                for mt in range(8):
                    for th in range(2):
                        sl = slice(512 * th, 512 * (th + 1))
                        ps_o = pa.tile([128, T], F32, tag="scores", bufs=2)
                        nc.tensor.matmul(
                            ps_o[:, 0:512], wo[:, 3, mt, :], ctx[:, 3, sl],
                            start=True, stop=True,
                        )
                        ysb = tr.tile([128, 512], F32, tag="ysb")
                        nc.vector.tensor_tensor(
                            ysb, ps_o[:, 0:512], y_acc[:, mt, sl], op=ALU.add
                        )
                        nc.sync.dma_start(
                            out=YT.ap()[128 * mt:128 * (mt + 1), sl], in_=ysb
                        )

        pw.release()
        pa.release()
        tr.release()
        pk.release()
        pc.release()


    nc.finalize()
    return nc


def _host_inputs(inputs):
    """Build the 8 per-core input maps from full inputs."""
    X = np.asarray(inputs["X"], dtype=np.float32)
    W_Q = np.asarray(inputs["W_Q"], dtype=np.float32)
    b_Q = np.asarray(inputs["b_Q"], dtype=np.float32)
    W_K = np.asarray(inputs["W_K"], dtype=np.float32)
    b_K = np.asarray(inputs["b_K"], dtype=np.float32)
    W_V = np.asarray(inputs["W_V"], dtype=np.float32)
    b_V = np.asarray(inputs["b_V"], dtype=np.float32)
    sink = np.asarray(inputs["sink_scalars"], dtype=np.float32)
    v_nulls = np.asarray(inputs["v_nulls"], dtype=np.float32)
    W_O = np.asarray(inputs["W_O"], dtype=np.float32)

    XT = np.ascontiguousarray(X[0].T)  # [C, T]

    # channel permutation (evens then odds) within each head's 64 channels
    perm64 = np.concatenate([np.arange(0, 64, 2), np.arange(1, 64, 2)])
    perm512 = (np.arange(8)[:, None] * 64 + perm64[None, :]).reshape(-1)

    # RoPE tables, matching reference float32 math
    invf = (1.0 / (10000.0 ** (np.arange(0, DH, 2, dtype=np.float32) / DH))).astype(
        np.float32
    )
    freqs = np.arange(T, dtype=np.float32)[:, None] * invf[None, :]  # [T, 32]
    cos32 = np.cos(freqs).T  # [32, T]
    sin32 = np.sin(freqs).T
    cos128 = np.tile(cos32, (4, 1)).astype(np.float16)
    sin128 = np.concatenate([-sin32, sin32, -sin32, sin32], axis=0).astype(np.float16)

    # swap matrix: out[p] = q[partner(p)]; lhsT[p', p] = 1 iff p' = partner(p)
    pswap = np.zeros((128, 128), dtype=np.float16)
    for p in range(128):
        partner = p + 32 if (p % 64) < 32 else p - 32
        pswap[partner, p] = 1.0

    # key_self selectors: sel[g][p, h] = 1 iff h == 2g + (p >= 64)
    sel = np.zeros((128, 4, 8), dtype=np.float16)
    for g in range(4):
        sel[0:64, g, 2 * g] = 1.0
        sel[64:128, g, 2 * g + 1] = 1.0

    in_maps = []
    for c in range(N_CORES):
        n, half = c // 2, c % 2
        qs = slice(512 * c, 512 * (c + 1))
        ks = slice(512 * half, 512 * (half + 1))
        heads = np.arange(8 * c, 8 * c + 8)
        sinks = sink[heads]  # [8]
        thr = np.tile((S * sinks).astype(np.float32)[None, :], (128, 1))
        tb = (S * (sinks + 1e-6)).astype(np.float32)[None, :]
        # vns[p, j]: head h = 2j + p//64, d = p%64 ; = S*sink_h*v_null[h]
        vn = v_nulls[n].reshape(N_HEAD, DH)  # base-head x d
        vns = np.zeros((64, 8), dtype=np.float32)
        for h in range(8):
            bh = (8 * half) + h  # base head index within branch
            vns[:, h] = S * sinks[h] * vn[bh]
        in_maps.append(
            {
                "XT": XT.astype(np.float16),
                "WQ": np.ascontiguousarray(W_Q[:, qs][:, perm512]).astype(np.float16),
                "BQ": np.ascontiguousarray(b_Q[qs][perm512])[None, :].astype(np.float16),
                "WK": np.ascontiguousarray(W_K[:, ks][:, perm512]).astype(np.float16),
                "BK": np.ascontiguousarray(b_K[ks][perm512])[None, :].astype(np.float16),
                "WV": np.ascontiguousarray(W_V[:, ks]).astype(np.float16),
                "BV": np.ascontiguousarray(b_V[ks])[None, :].astype(np.float16),
                "WO": np.ascontiguousarray(0.25 * W_O[n, ks, :]),
                "COS": cos128,
                "SIN": sin128,
                "PSW": pswap,
                "SEL": sel,
                "THR": thr,
                "TB": tb,
                "VNS": vns,
                "ONES": np.ones((1, 512), dtype=np.float16),
            }
        )
    return in_maps


def kernel(**inputs) -> np.ndarray:
    from concourse.bass_utils import run_bass_kernel_spmd

    in_maps = _host_inputs(inputs)
    if _NC_CACHE[0] is None:
        _NC_CACHE[0] = _build_nc()
    nc = _NC_CACHE[0]
    trace = bool(os.environ.get("KBENCH_TRACE"))
    res = run_bass_kernel_spmd(
        nc, in_maps, core_ids=list(range(N_CORES)), trace=trace
    )
    LAST_RESULT[0] = res
    if trace and res.exec_time_ns is not None:
        print(f"HW exec time: {res.exec_time_ns} ns")

    W_O_bias = np.asarray(inputs["W_O_bias"], dtype=np.float32)
    y = np.zeros((T, D_MODEL), dtype=np.float32)
    for r in res.results:
        y += r["YT"].T
    y += W_O_bias.mean(axis=0)[None, :]
    return y[None, :, :]


# revision 12
# speedup vs baseline: 1.0101x; 1.0101x over previous
"""Trainium2 Bass kernel for nn_Attention_65609920414302 (sparse multi-branch attention).

Sharding: 64 total heads (4 branches x 16 sub-heads) split as 8 heads per core
(core c = branch c//2, base-head half c%2). Each core computes Q/K/V projections
for its heads, RoPE, causal thresholded-softplus attention, and a partial W_O
matmul; the host sums the 8 partial outputs.

Math rescaling used on device (S = pi/sqrt(3)):
  reference w_sig = w*sigmoid(S*w) with w = softplus(scores*m), thresholded at sink.
  device   W = silu(S*w) = S*w_sig, thresholded at S*sink,
  probs    = W / (sum_s W + S*(sink+1e-6)),  sink term = S*sink / (...).
The S factors cancel exactly.

Layouts (per core):
  XT   [C=1024, T=1024]   X transposed (f32r)
  Q^T/K^T tiles [128ch, 4, 1024t] (fp16 after RoPE); channel order per head is
      [evens(32) | odds(32)] (host permutes W_Q/W_K columns) so RoPE is a
      32-row swap + cos/sin elementwise.
  V    [128t, 8tt, 8head, 65] (fp16), col 64 = ones (gives sum_s W via PV matmul)
  scores^T tiles [128 s, L] in PSUM per (head, s_tile), L = 1024-128*i (causal)
  Wbuf [128 s, 8 head, 4608] fp16: concatenated valid t-ranges per s_tile
  ctx  [128ch, 4, 1024t] f32r -> W_O (f32r) -> YT [1024 d, 1024 t] partial out
"""

import math
import os
import numpy as np

D_MODEL = 1024
N_HEAD = 16
N_BR = 4
DH = 64
H_TOT = 64
T = 1024
S = math.pi / math.sqrt(3.0)
ATTNSCALE = DH ** -0.5
N_CORES = 8
HPC = 8          # heads per core
KT = 8           # C // 128 contraction tiles
L_LIST = [T - 128 * i for i in range(8)]
O_LIST = [sum(L_LIST[:i]) for i in range(8)]
W_COLS = sum(L_LIST)  # 4608

_NC_CACHE = [None]
LAST_RESULT = [None]  # stash for test harness (exec_time_ns etc.)


def _build_nc():
    import concourse.bass as bass
    from concourse import bacc
    import concourse.mybir as mybir
    import concourse.tile as tile
    from concourse.masks import make_identity

    F32 = mybir.dt.float32
    F32R = mybir.dt.float32r
    F16 = mybir.dt.float16
    AF = mybir.ActivationFunctionType
    ALU = mybir.AluOpType

    nc = bacc.Bacc(None, target_bir_lowering=False, debug=False)

    # ---- DRAM parameters (per-core data; same program on all cores) ----
    XT = nc.declare_dram_parameter("XT", [D_MODEL, T], F16, isOutput=False)
    WQ = nc.declare_dram_parameter("WQ", [D_MODEL, 512], F16, isOutput=False)
    BQ = nc.declare_dram_parameter("BQ", [1, 512], F16, isOutput=False)
    WK = nc.declare_dram_parameter("WK", [D_MODEL, 512], F16, isOutput=False)
    BK = nc.declare_dram_parameter("BK", [1, 512], F16, isOutput=False)
    WV = nc.declare_dram_parameter("WV", [D_MODEL, 512], F16, isOutput=False)
    BV = nc.declare_dram_parameter("BV", [1, 512], F16, isOutput=False)
    WO = nc.declare_dram_parameter("WO", [512, D_MODEL], F32R, isOutput=False)
    COS = nc.declare_dram_parameter("COS", [128, T], F16, isOutput=False)
    SIN = nc.declare_dram_parameter("SIN", [128, T], F16, isOutput=False)
    PSW = nc.declare_dram_parameter("PSW", [128, 128], F16, isOutput=False)
    SEL = nc.declare_dram_parameter("SEL", [128, 4, 8], F16, isOutput=False)
    THR = nc.declare_dram_parameter("THR", [128, 8], F32, isOutput=False)
    TB = nc.declare_dram_parameter("TB", [1, 8], F32, isOutput=False)
    VNS = nc.declare_dram_parameter("VNS", [64, 8], F32, isOutput=False)
    ONES = nc.declare_dram_parameter("ONES", [1, 512], F16, isOutput=False)
    YT = nc.declare_dram_parameter("YT", [D_MODEL, T], F32, isOutput=True)

    with tile.TileContext(nc) as tc:
        pc = tc.alloc_tile_pool(name="const", bufs=1)
        pk = tc.alloc_tile_pool(name="keep", bufs=1)
        tr = tc.alloc_tile_pool(name="trans", bufs=2)
        pp = tc.alloc_tile_pool(name="proj", bufs=1)
        pj = tc.alloc_tile_pool(name="psproj", bufs=1, space="PSUM")

        # ---- constants / weights into SBUF ----
        cos_sb = pc.tile([128, T], F16)
        sin_sb = pc.tile([128, T], F16)
        psw_sb = pc.tile([128, 128], F16)
        sel_sb = pc.tile([128, 4, 8], F16)
        thr_sb = pc.tile([128, 8], F32)
        tb_sb = pc.tile([1, 8], F32)
        vns_sb = pc.tile([64, 8], F32)
        ident = pc.tile([128, 128], F32)
        ones_r = pc.tile([1, 512], F16)
        m_colsb = pc.tile([128, 8, 8], F32)
        m_all = pc.tile([8, T], F32)
        ksc = pc.tile([8, T], F32)

        nc.sync.dma_start(out=cos_sb, in_=COS.ap())
        nc.sync.dma_start(out=sin_sb, in_=SIN.ap())
        nc.sync.dma_start(out=psw_sb, in_=PSW.ap())
        nc.sync.dma_start(out=sel_sb, in_=SEL.ap())
        nc.sync.dma_start(out=thr_sb, in_=THR.ap())
        nc.sync.dma_start(out=tb_sb, in_=TB.ap())
        nc.sync.dma_start(out=vns_sb, in_=VNS.ap())
        make_identity(nc, ident)
        nc.sync.dma_start(out=ones_r, in_=ONES.ap())

        xt = pp.tile([128, KT, T], F16)
        wq = pp.tile([128, KT, 4, 128], F16)
        wk = pp.tile([128, KT, 4, 128], F16)
        wv = pp.tile([128, KT, 512], F16)
        bq = pp.tile([1, 512], F16)
        bk = pp.tile([1, 512], F16)
        bv = pp.tile([1, 512], F16)
        xt_src = XT.ap().rearrange("(kt p) t -> p kt t", p=128)
        wk_src = WK.ap().rearrange("(kt p) (mt m) -> p kt mt m", p=128, m=128)
        for kt in range(KT):
            nc.sync.dma_start(out=xt[:, kt, :], in_=xt_src[:, kt, :])
            nc.sync.dma_start(out=wk[:, kt, :, :], in_=wk_src[:, kt, :, :])
        nc.sync.dma_start(
            out=wq, in_=WQ.ap().rearrange("(kt p) (mt m) -> p kt mt m", p=128, m=128)
        )
        nc.sync.dma_start(out=wv, in_=WV.ap().rearrange("(kt p) v -> p kt v", p=128))
        nc.sync.dma_start(out=bq, in_=BQ.ap())
        nc.sync.dma_start(out=bk, in_=BK.ap())
        nc.sync.dma_start(out=bv, in_=BV.ap())

        wo = pk.tile([128, 4, 8, 128], F32R)
        nc.sync.dma_start(
            out=wo, in_=WO.ap().rearrange("(ct p) (mt m) -> p ct mt m", p=128, m=128)
        )

        qrope = pk.tile([128, 4, T], F16)
        krope = pk.tile([128, 4, T], F16)
        vstore = pk.tile([128, 8, HPC, 65], F16)
        ctx = pk.tile([128, 4, T], F32R)
        nc.vector.memset(vstore[:, :, :, 64:65], 1.0)

        # ---- projection + rope for K and Q ----
        def proj_rope(w_t, b_t, out_t):
            for g in range(4):
                ps = pj.tile([128, T], F32, tag="projps", bufs=2)
                for th in range(2):
                    sl = slice(512 * th, 512 * (th + 1))
                    for kt in range(KT):
                        nc.tensor.matmul(
                            ps[:, sl], w_t[:, kt, g, :], xt[:, kt, sl],
                            start=(kt == 0), stop=False,
                        )
                    nc.tensor.matmul(
                        ps[:, sl], b_t[0:1, 128 * g:128 * (g + 1)], ones_r,
                        start=False, stop=True,
                    )
                qsb = tr.tile([128, T], F16, tag="qsb")
                nc.vector.tensor_copy(qsb, ps)
                sw = pj.tile([128, T], F32, tag="swapps")
                for th in range(2):
                    sl = slice(512 * th, 512 * (th + 1))
                    nc.tensor.matmul(sw[:, sl], psw_sb, qsb[:, sl], start=True, stop=True)
                t1 = tr.tile([128, T], F16, tag="t1")
                nc.vector.tensor_tensor(t1, qsb, cos_sb, op=ALU.mult)
                t2 = tr.tile([128, T], F16, tag="t2")
                nc.vector.tensor_tensor(t2, sw, sin_sb, op=ALU.mult)
                nc.gpsimd.tensor_tensor(out_t[:, g, :], t1, t2, op=ALU.add)

        proj_rope(wk, bk, krope)

        # ---- key_self -> m (scale columns for exp) ----
        ks_ps = pj.tile([8, T], F32, tag="swapps")
        for g in range(4):
            k2 = tr.tile([128, T], F16, tag="k2")
            nc.vector.tensor_tensor(k2, krope[:, g, :], krope[:, g, :], op=ALU.mult)
            for th in range(2):
                sl = slice(512 * th, 512 * (th + 1))
                nc.tensor.matmul(
                    ks_ps[:, sl], sel_sb[:, g, :], k2[:, sl],
                    start=(g == 0), stop=(g == 3),
                )
        nc.vector.tensor_scalar_max(ksc, ks_ps, 1e-6)
        nc.vector.reciprocal_approx_fast(ksc, ksc)
        # m = ATTNSCALE / sqrt(key_self) = sqrt(recip / DH)
        nc.scalar.activation(m_all, ksc, AF.Sqrt, scale=1.0 / DH)
        for i in range(8):
            mt_ps = pj.tile([128, 8], F32, tag="swapps")
            nc.tensor.transpose(mt_ps, m_all[:, 128 * i:128 * (i + 1)], ident[0:8, 0:8])
            nc.vector.tensor_copy(m_colsb[:, i, :], mt_ps)

        proj_rope(wq, bq, qrope)

        # ---- V projection (t on partitions) ----
        for tt_i in range(8):
            psv = pj.tile([128, T], F32, tag="projps", bufs=2)
            for kt in range(KT):
                nc.tensor.matmul(
                    psv[:, 0:512], xt[:, kt, 128 * tt_i:128 * (tt_i + 1)],
                    wv[:, kt, :], start=(kt == 0), stop=False,
                )
            nc.tensor.matmul(
                psv[:, 0:512], ones_r[0:1, 0:128], bv, start=False, stop=True
            )
            nc.vector.tensor_copy(
                vstore[:, tt_i, :, 0:64],
                psv[:, 0:512].rearrange("p (h d) -> p h d", d=64),
            )

        # ---- attention ----
        pp.release()
        pj.release()
        pa = tc.alloc_tile_pool(name="psattn", bufs=1, space="PSUM")
        pw = tc.alloc_tile_pool(name="wbuf", bufs=1)
        wbuf = pw.tile([128, HPC, W_COLS], F16)

        # scores + exp + ln + silu + threshold + PV, in 2 waves of 4 heads.
        # Within a wave: all exps (one table set), then all lns, then silus
        # (dep-enforced ordering keeps ACT table loads to one per phase).
        from concourse.tile import add_dep_helper

        prev_wave_silu = []
        y_acc = pw.tile([128, 8, T], F16)
        for wi, pairs in enumerate([(0, 1), (2, 3)]):
            exp_insts = []
            for j in pairs:
                for i in range(8):
                    t0 = 128 * i
                    L = L_LIST[i]
                    pss = []
                    for u in range(2):
                        h = 2 * j + u
                        g, r0 = h // 2, 64 * (h % 2)
                        ps_s = pa.tile([128, T], F32, tag="scores", bufs=2)
                        for c0 in range(0, L, 512):
                            c1 = min(c0 + 512, L)
                            nc.tensor.matmul(
                                ps_s[:, c0:c1],
                                krope[r0:r0 + 64, g, t0:t0 + 128],
                                qrope[r0:r0 + 64, g, t0 + c0:t0 + c1],
                                start=True, stop=True,
                            )
                        pss.append((h, ps_s))
                    for h, ps_s in pss:
                        o = O_LIST[i]
                        e = nc.scalar.activation(
                            wbuf[:, h, o:o + L], ps_s[:, 0:L], AF.Exp,
                            scale=m_colsb[:, i, h:h + 1],
                        )
                        for si in prev_wave_silu:
                            add_dep_helper(e.ins, si.ins, sync=False,
                                           reason="act table phase order")
                        exp_insts.append(e)
                        # zero the upper-triangular part of the diagonal block
                        nc.gpsimd.affine_select(
                            out=wbuf[:, h, o:o + 128], in_=wbuf[:, h, o:o + 128],
                            compare_op=ALU.is_ge, fill=0.0, base=0,
                            pattern=[[1, 128]], channel_multiplier=-1,
                        )
            ln_insts = []
            for j in pairs:
                for u in range(2):
                    h = 2 * j + u
                    ln = nc.scalar.activation(
                        wbuf[:, h, :], wbuf[:, h, :], AF.Ln, bias=1.0
                    )
                    for e in exp_insts:
                        add_dep_helper(ln.ins, e.ins, sync=False,
                                       reason="act table phase order")
                    ln_insts.append((h, ln))
            wave_silu = []
            for h, _ln in ln_insts:
                si = nc.scalar.activation(
                    wbuf[:, h, :], wbuf[:, h, :], AF.Silu, scale=S
                )
                for _h2, l2 in ln_insts:
                    add_dep_helper(si.ins, l2.ins, sync=False,
                                   reason="act table phase order")
                wave_silu.append(si)
                # threshold: w = (w >= thr) * w   (mask on 4x path, mult on 2x)
                msk = tr.tile([128, W_COLS], F16, tag="msk", bufs=1)
                nc.vector.tensor_scalar(
                    msk, wbuf[:, h, :], thr_sb[:, h:h + 1], None, op0=ALU.is_ge
                )
                nc.vector.tensor_tensor(
                    wbuf[:, h, :], wbuf[:, h, :], msk, op=ALU.mult
                )
                ps_pv = pa.tile([65, T], F32, tag="pv", bufs=2)
                for i in range(8):
                    t0 = 128 * i
                    o = O_LIST[i]
                    chunks = []
                    if t0 < 512:
                        chunks.append((t0, 512, 3))
                        chunks.append((512, T, 7))
                    else:
                        chunks.append((t0, T, 7))
                    for (a, b, last_i) in chunks:
                        nc.tensor.matmul(
                            ps_pv[:, a:b],
                            vstore[:, i, h, :],
                            wbuf[:, h, o + (a - t0):o + (b - t0)],
                            start=(i == 0), stop=(i == last_i),
                        )
                tp = tr.tile([1, T], F32, tag="tp")
                nc.vector.tensor_scalar_add(tp, ps_pv[64:65, :], tb_sb[0:1, h:h + 1])
                nc.vector.reciprocal_approx_fast(tp, tp)
                gb = tr.tile([64, T], F32, tag="gb")
                nc.gpsimd.partition_broadcast(gb, tp, channels=64)
                r0 = 64 * (h % 2)
                nc.vector.scalar_tensor_tensor(
                    out=ctx[r0:r0 + 64, h // 2, :], in0=ps_pv[0:64, :],
                    scalar=vns_sb[:, h:h + 1], in1=gb,
                    op0=ALU.add, op1=ALU.mult,
                )
            prev_wave_silu = wave_silu

            # W_O partial for this wave's ctx tiles; last wave adds the
            # accumulated first-wave partials and writes out.
            cts = [2 * j_ for j_ in pairs] if False else None
            if wi == 0:
                for mt in range(8):
                    for th in range(2):
                        sl = slice(512 * th, 512 * (th + 1))
                        ps_o = pa.tile([128, T], F32, tag="scores", bufs=2)
                        for ci, ct in enumerate((0, 1)):
                            nc.tensor.matmul(
                                ps_o[:, 0:512], wo[:, ct, mt, :], ctx[:, ct, sl],
                                start=(ci == 0), stop=(ci == 1),
                            )
                        nc.vector.tensor_copy(y_acc[:, mt, sl], ps_o[:, 0:512])
            else:
                for mt in range(8):
                    for th in range(2):
                        sl = slice(512 * th, 512 * (th + 1))
                        ps_o = pa.tile([128, T], F32, tag="scores", bufs=2)
                        for ci, ct in enumerate((2, 3)):
                            nc.tensor.matmul(
                                ps_o[:, 0:512], wo[:, ct, mt, :], ctx[:, ct, sl],
                                start=(ci == 0), stop=(ci == 1),
                            )
                        ysb = tr.tile([128, 512], F32, tag="ysb")
                        nc.vector.tensor_tensor(
                            ysb, ps_o[:, 0:512], y_acc[:, mt, sl], op=ALU.add
                        )
                        nc.sync.dma_start(
                            out=YT.ap()[128 * mt:128 * (mt + 1), sl], in_=ysb
                        )

        pw.release()
        pa.release()
        tr.release()
        pk.release()
        pc.release()


    nc.finalize()
    return nc


def _host_inputs(inputs):
    """Build the 8 per-core input maps from full inputs."""
    X = np.asarray(inputs["X"], dtype=np.float32)
    W_Q = np.asarray(inputs["W_Q"], dtype=np.float32)
    b_Q = np.asarray(inputs["b_Q"], dtype=np.float32)
    W_K = np.asarray(inputs["W_K"], dtype=np.float32)
    b_K = np.asarray(inputs["b_K"], dtype=np.float32)
    W_V = np.asarray(inputs["W_V"], dtype=np.float32)
    b_V = np.asarray(inputs["b_V"], dtype=np.float32)
    sink = np.asarray(inputs["sink_scalars"], dtype=np.float32)
    v_nulls = np.asarray(inputs["v_nulls"], dtype=np.float32)
    W_O = np.asarray(inputs["W_O"], dtype=np.float32)

    XT = np.ascontiguousarray(X[0].T)  # [C, T]

    # channel permutation (evens then odds) within each head's 64 channels
    perm64 = np.concatenate([np.arange(0, 64, 2), np.arange(1, 64, 2)])
    perm512 = (np.arange(8)[:, None] * 64 + perm64[None, :]).reshape(-1)

    # RoPE tables, matching reference float32 math
    invf = (1.0 / (10000.0 ** (np.arange(0, DH, 2, dtype=np.float32) / DH))).astype(
        np.float32
    )
    freqs = np.arange(T, dtype=np.float32)[:, None] * invf[None, :]  # [T, 32]
    cos32 = np.cos(freqs).T  # [32, T]
    sin32 = np.sin(freqs).T
    cos128 = np.tile(cos32, (4, 1)).astype(np.float16)
    sin128 = np.concatenate([-sin32, sin32, -sin32, sin32], axis=0).astype(np.float16)

    # swap matrix: out[p] = q[partner(p)]; lhsT[p', p] = 1 iff p' = partner(p)
    pswap = np.zeros((128, 128), dtype=np.float16)
    for p in range(128):
        partner = p + 32 if (p % 64) < 32 else p - 32
        pswap[partner, p] = 1.0

    # key_self selectors: sel[g][p, h] = 1 iff h == 2g + (p >= 64)
    sel = np.zeros((128, 4, 8), dtype=np.float16)
    for g in range(4):
        sel[0:64, g, 2 * g] = 1.0
        sel[64:128, g, 2 * g + 1] = 1.0

    in_maps = []
    for c in range(N_CORES):
        n, half = c // 2, c % 2
        qs = slice(512 * c, 512 * (c + 1))
        ks = slice(512 * half, 512 * (half + 1))
        heads = np.arange(8 * c, 8 * c + 8)
        sinks = sink[heads]  # [8]
        thr = np.tile((S * sinks).astype(np.float32)[None, :], (128, 1))
        tb = (S * (sinks + 1e-6)).astype(np.float32)[None, :]
        # vns[p, j]: head h = 2j + p//64, d = p%64 ; = S*sink_h*v_null[h]
        vn = v_nulls[n].reshape(N_HEAD, DH)  # base-head x d
        vns = np.zeros((64, 8), dtype=np.float32)
        for h in range(8):
            bh = (8 * half) + h  # base head index within branch
            vns[:, h] = S * sinks[h] * vn[bh]
        in_maps.append(
            {
                "XT": XT.astype(np.float16),
                "WQ": np.ascontiguousarray(W_Q[:, qs][:, perm512]).astype(np.float16),
                "BQ": np.ascontiguousarray(b_Q[qs][perm512])[None, :].astype(np.float16),
                "WK": np.ascontiguousarray(W_K[:, ks][:, perm512]).astype(np.float16),
                "BK": np.ascontiguousarray(b_K[ks][perm512])[None, :].astype(np.float16),
                "WV": np.ascontiguousarray(W_V[:, ks]).astype(np.float16),
                "BV": np.ascontiguousarray(b_V[ks])[None, :].astype(np.float16),
                "WO": np.ascontiguousarray(0.25 * W_O[n, ks, :]),
                "COS": cos128,
                "SIN": sin128,
                "PSW": pswap,
                "SEL": sel,
                "THR": thr,
                "TB": tb,
                "VNS": vns,
                "ONES": np.ones((1, 512), dtype=np.float16),
            }
        )
    return in_maps


def kernel(**inputs) -> np.ndarray:
    from concourse.bass_utils import run_bass_kernel_spmd

    in_maps = _host_inputs(inputs)
    if _NC_CACHE[0] is None:
        _NC_CACHE[0] = _build_nc()
    nc = _NC_CACHE[0]
    trace = bool(os.environ.get("KBENCH_TRACE"))
    res = run_bass_kernel_spmd(
        nc, in_maps, core_ids=list(range(N_CORES)), trace=trace
    )
    LAST_RESULT[0] = res
    if trace and res.exec_time_ns is not None:
        print(f"HW exec time: {res.exec_time_ns} ns")

    W_O_bias = np.asarray(inputs["W_O_bias"], dtype=np.float32)
    y = np.zeros((T, D_MODEL), dtype=np.float32)
    for r in res.results:
        y += r["YT"].T
    y += W_O_bias.mean(axis=0)[None, :]
    return y[None, :, :]


# revision 15
# speedup vs baseline: 1.0323x; 1.0220x over previous
"""Trainium2 Bass kernel for nn_Attention_65609920414302 (sparse multi-branch attention).

Sharding: 64 total heads (4 branches x 16 sub-heads) split as 8 heads per core
(core c = branch c//2, base-head half c%2). Each core computes Q/K/V projections
for its heads, RoPE, causal thresholded-softplus attention, and a partial W_O
matmul; the host sums the 8 partial outputs.

Math rescaling used on device (S = pi/sqrt(3)):
  reference w_sig = w*sigmoid(S*w) with w = softplus(scores*m), thresholded at sink.
  device   W = silu(S*w) = S*w_sig, thresholded at S*sink,
  probs    = W / (sum_s W + S*(sink+1e-6)),  sink term = S*sink / (...).
The S factors cancel exactly. softplus is composed as ln(1 + exp(m*x)) because
this toolchain has no softplus ACT table; exp/ln/silu phases are ordered with
explicit deps so each wave costs exactly 3 ACT table loads.

Pipeline: 4 waves of 1 head-pair each. Per wave: scores (PE, fp16) -> exp (ACT)
-> causal mask (gpsimd) -> ln (ACT) -> silu (ACT) -> threshold (DVE) -> PV (PE)
-> 1/total (DVE approx recip) -> broadcast (gpsimd) -> context normalize (DVE).
W_O runs in two halves (after waves 1 and 3) accumulating through an fp16 SBUF
buffer so most of it overlaps the ACT phases.
"""

import math
import os
import numpy as np

D_MODEL = 1024
N_HEAD = 16
N_BR = 4
DH = 64
H_TOT = 64
T = 1024
S = math.pi / math.sqrt(3.0)
ATTNSCALE = DH ** -0.5
N_CORES = 8
HPC = 8          # heads per core
KT = 8           # C // 128 contraction tiles
L_LIST = [T - 128 * i for i in range(8)]
O_LIST = [sum(L_LIST[:i]) for i in range(8)]
W_COLS = sum(L_LIST)  # 4608

_NC_CACHE = [None]
LAST_RESULT = [None]  # stash for test harness (exec_time_ns etc.)


def _build_nc():
    import concourse.bass as bass
    from concourse import bacc
    import concourse.mybir as mybir
    import concourse.tile as tile
    from concourse.tile import add_dep_helper
    from concourse.masks import make_identity

    F32 = mybir.dt.float32
    F32R = mybir.dt.float32r
    F16 = mybir.dt.float16
    AF = mybir.ActivationFunctionType
    ALU = mybir.AluOpType

    nc = bacc.Bacc(None, target_bir_lowering=False, debug=False)

    # ---- DRAM parameters (per-core data; same program on all cores) ----
    XT = nc.declare_dram_parameter("XT", [D_MODEL, T], F16, isOutput=False)
    WQ = nc.declare_dram_parameter("WQ", [D_MODEL, 512], F16, isOutput=False)
    BQ = nc.declare_dram_parameter("BQ", [1, 512], F16, isOutput=False)
    WK = nc.declare_dram_parameter("WK", [D_MODEL, 512], F16, isOutput=False)
    BK = nc.declare_dram_parameter("BK", [1, 512], F16, isOutput=False)
    WV = nc.declare_dram_parameter("WV", [D_MODEL, 512], F16, isOutput=False)
    BV = nc.declare_dram_parameter("BV", [1, 512], F16, isOutput=False)
    WO = nc.declare_dram_parameter("WO", [512, D_MODEL], F32R, isOutput=False)
    COS = nc.declare_dram_parameter("COS", [128, T], F16, isOutput=False)
    SIN = nc.declare_dram_parameter("SIN", [128, T], F16, isOutput=False)
    PSW = nc.declare_dram_parameter("PSW", [128, 128], F16, isOutput=False)
    SEL = nc.declare_dram_parameter("SEL", [128, 4, 8], F16, isOutput=False)
    THR = nc.declare_dram_parameter("THR", [128, 8], F32, isOutput=False)
    TB = nc.declare_dram_parameter("TB", [1, 8], F32, isOutput=False)
    VNS = nc.declare_dram_parameter("VNS", [64, 8], F32, isOutput=False)
    ONES = nc.declare_dram_parameter("ONES", [1, 512], F16, isOutput=False)
    YT = nc.declare_dram_parameter("YT", [D_MODEL, T], F32, isOutput=True)

    with tile.TileContext(nc) as tc:
        pc = tc.alloc_tile_pool(name="const", bufs=1)
        pk = tc.alloc_tile_pool(name="keep", bufs=1)
        tr = tc.alloc_tile_pool(name="trans", bufs=2)
        pw = tc.alloc_tile_pool(name="wbuf", bufs=1)
        pp2 = tc.alloc_tile_pool(name="projxv", bufs=1)
        pp1 = tc.alloc_tile_pool(name="projqk", bufs=1)
        pj = tc.alloc_tile_pool(name="psproj", bufs=1, space="PSUM")

        # ---- constants ----
        cos_sb = pc.tile([128, T], F16)
        sin_sb = pc.tile([128, T], F16)
        psw_sb = pc.tile([128, 128], F16)
        sel_sb = pc.tile([128, 4, 8], F16)
        thr_sb = pc.tile([128, 8], F32)
        tb_sb = pc.tile([1, 8], F32)
        vns_sb = pc.tile([64, 8], F32)
        ident = pc.tile([128, 128], F32)
        ones_r = pc.tile([1, 512], F16)
        m_colsb = pc.tile([128, 8, 8], F32)
        m_all = pc.tile([8, T], F32)

        nc.sync.dma_start(out=cos_sb, in_=COS.ap())
        nc.sync.dma_start(out=sin_sb, in_=SIN.ap())
        nc.sync.dma_start(out=psw_sb, in_=PSW.ap())
        nc.sync.dma_start(out=sel_sb, in_=SEL.ap())
        nc.sync.dma_start(out=thr_sb, in_=THR.ap())
        nc.sync.dma_start(out=tb_sb, in_=TB.ap())
        nc.sync.dma_start(out=vns_sb, in_=VNS.ap())
        make_identity(nc, ident)
        nc.sync.dma_start(out=ones_r, in_=ONES.ap())

        # ---- weights ----
        xt = pp2.tile([128, KT, T], F16)
        wv = pp2.tile([128, KT, 512], F16)
        bv = pp2.tile([1, 512], F16)
        wq = pp1.tile([128, KT, 4, 128], F16)
        wk = pp1.tile([128, KT, 4, 128], F16)
        bq = pp1.tile([1, 512], F16)
        bk = pp1.tile([1, 512], F16)
        xt_src = XT.ap().rearrange("(kt p) t -> p kt t", p=128)
        wk_src = WK.ap().rearrange("(kt p) (mt m) -> p kt mt m", p=128, m=128)
        for kt in range(KT):
            nc.sync.dma_start(out=xt[:, kt, :], in_=xt_src[:, kt, :])
            nc.sync.dma_start(out=wk[:, kt, :, :], in_=wk_src[:, kt, :, :])
        nc.sync.dma_start(
            out=wq, in_=WQ.ap().rearrange("(kt p) (mt m) -> p kt mt m", p=128, m=128)
        )
        nc.sync.dma_start(out=wv, in_=WV.ap().rearrange("(kt p) v -> p kt v", p=128))
        nc.sync.dma_start(out=bq, in_=BQ.ap())
        nc.sync.dma_start(out=bk, in_=BK.ap())
        nc.sync.dma_start(out=bv, in_=BV.ap())

        wo = pk.tile([128, 4, 8, 128], F32R)
        nc.sync.dma_start(
            out=wo, in_=WO.ap().rearrange("(ct p) (mt m) -> p ct mt m", p=128, m=128)
        )

        qrope = pk.tile([128, 4, T], F16)
        krope = pk.tile([128, 4, T], F16)
        vstore = pk.tile([128, 8, HPC, 65], F16)
        ctx = pk.tile([128, 4, T], F32R)
        y_acc = pk.tile([128, 8, T], F16)
        nc.vector.memset(vstore[:, :, :, 64:65], 1.0)

        # ---- projection + rope for K and Q ----
        def proj_rope(w_t, b_t, out_t):
            for g in range(4):
                ps = pj.tile([128, T], F32, tag="projps", bufs=2)
                for th in range(2):
                    sl = slice(512 * th, 512 * (th + 1))
                    for kt in range(KT):
                        nc.tensor.matmul(
                            ps[:, sl], w_t[:, kt, g, :], xt[:, kt, sl],
                            start=(kt == 0), stop=False,
                        )
                    nc.tensor.matmul(
                        ps[:, sl], b_t[0:1, 128 * g:128 * (g + 1)], ones_r,
                        start=False, stop=True,
                    )
                qsb = tr.tile([128, T], F16, tag="qsb")
                nc.vector.tensor_copy(qsb, ps)
                sw = pj.tile([128, T], F32, tag="swapps")
                for th in range(2):
                    sl = slice(512 * th, 512 * (th + 1))
                    nc.tensor.matmul(sw[:, sl], psw_sb, qsb[:, sl], start=True,
                                     stop=True)
                t1 = tr.tile([128, T], F16, tag="t1")
                nc.vector.tensor_tensor(t1, qsb, cos_sb, op=ALU.mult)
                t2 = tr.tile([128, T], F16, tag="t2")
                nc.vector.tensor_tensor(t2, sw, sin_sb, op=ALU.mult)
                nc.gpsimd.tensor_tensor(out_t[:, g, :], t1, t2, op=ALU.add)

        proj_rope(wk, bk, krope)

        # ---- key_self -> m (per-key scale folded into the exp pass) ----
        ks_ps = pj.tile([8, T], F32, tag="swapps")
        for g in range(4):
            k2 = tr.tile([128, T], F16, tag="k2", bufs=1)
            nc.vector.tensor_tensor(k2, krope[:, g, :], krope[:, g, :], op=ALU.mult)
            for th in range(2):
                sl = slice(512 * th, 512 * (th + 1))
                nc.tensor.matmul(
                    ks_ps[:, sl], sel_sb[:, g, :], k2[:, sl],
                    start=(g == 0), stop=(g == 3),
                )
        nc.vector.tensor_scalar_max(m_all, ks_ps, 1e-6)
        nc.vector.reciprocal_approx_fast(m_all, m_all)
        # m = ATTNSCALE / sqrt(key_self) = sqrt(recip / DH)
        nc.scalar.activation(m_all, m_all, AF.Sqrt, scale=1.0 / DH)
        for i in range(8):
            mt_ps = pj.tile([128, 8], F32, tag="swapps")
            nc.tensor.transpose(mt_ps, m_all[:, 128 * i:128 * (i + 1)],
                                ident[0:8, 0:8])
            nc.vector.tensor_copy(m_colsb[:, i, :], mt_ps)

        proj_rope(wq, bq, qrope)
        pp1.release()
        pj.release()

        pa = tc.alloc_tile_pool(name="psattn", bufs=1, space="PSUM")

        # ---- attention: 4 waves of one head-pair ----
        prev_wave_silu = []
        for wi in range(4):
            j = wi
            wbuf = pw.tile([128, 2, W_COLS], F16, tag="wbuf", bufs=2)
            exp_insts = []
            for i in range(8):
                t0 = 128 * i
                L = L_LIST[i]
                pss = []
                for u in range(2):
                    h = 2 * j + u
                    g, r0 = h // 2, 64 * (h % 2)
                    ps_s = pa.tile([128, T], F32, tag="scores", bufs=2)
                    for c0 in range(0, L, 512):
                        c1 = min(c0 + 512, L)
                        nc.tensor.matmul(
                            ps_s[:, c0:c1],
                            krope[r0:r0 + 64, g, t0:t0 + 128],
                            qrope[r0:r0 + 64, g, t0 + c0:t0 + c1],
                            start=True, stop=True,
                        )
                    pss.append((h, u, ps_s))
                for h, u, ps_s in pss:
                    o = O_LIST[i]
                    e = nc.scalar.activation(
                        wbuf[:, u, o:o + L], ps_s[:, 0:L], AF.Exp,
                        scale=m_colsb[:, i, h:h + 1],
                    )
                    for si in prev_wave_silu:
                        add_dep_helper(e.ins, si.ins, sync=False,
                                       reason="act table phase order")
                    exp_insts.append(e)
                    # zero the upper-triangular part of the diagonal block
                    nc.gpsimd.affine_select(
                        out=wbuf[:, u, o:o + 128], in_=wbuf[:, u, o:o + 128],
                        compare_op=ALU.is_ge, fill=0.0, base=0,
                        pattern=[[1, 128]], channel_multiplier=-1,
                    )

            if wi == 0:
                # V projection (t on partitions), overlapping the first exp phase
                for tt_i in range(8):
                    psv = pa.tile([128, T], F32, tag="scores", bufs=2)
                    for kt in range(KT):
                        nc.tensor.matmul(
                            psv[:, 0:512], xt[:, kt, 128 * tt_i:128 * (tt_i + 1)],
                            wv[:, kt, :], start=(kt == 0), stop=False,
                        )
                    nc.tensor.matmul(
                        psv[:, 0:512], ones_r[0:1, 0:128], bv, start=False, stop=True
                    )
                    nc.vector.tensor_copy(
                        vstore[:, tt_i, :, 0:64],
                        psv[:, 0:512].rearrange("p (h d) -> p h d", d=64),
                    )
                pp2.release()

            ln_insts = []
            for u in range(2):
                h = 2 * j + u
                ln = nc.scalar.activation(
                    wbuf[:, u, :], wbuf[:, u, :], AF.Ln, bias=1.0
                )
                for e in exp_insts:
                    add_dep_helper(ln.ins, e.ins, sync=False,
                                   reason="act table phase order")
                ln_insts.append((h, u, ln))
            wave_silu = []
            for h, u, _ln in ln_insts:
                si = nc.scalar.activation(
                    wbuf[:, u, :], wbuf[:, u, :], AF.Silu, scale=S
                )
                for _h2, _u2, l2 in ln_insts:
                    add_dep_helper(si.ins, l2.ins, sync=False,
                                   reason="act table phase order")
                wave_silu.append(si)
                # threshold: w = (w >= thr) * w
                nc.vector.scalar_tensor_tensor(
                    out=wbuf[:, u, :], in0=wbuf[:, u, :],
                    scalar=thr_sb[:, h:h + 1], in1=wbuf[:, u, :],
                    op0=ALU.is_ge, op1=ALU.mult,
                )
                ps_pv = pa.tile([65, T], F32, tag="pv", bufs=2)
                for i in range(8):
                    t0 = 128 * i
                    o = O_LIST[i]
                    chunks = []
                    if t0 < 512:
                        chunks.append((t0, 512, 3))
                        chunks.append((512, T, 7))
                    else:
                        chunks.append((t0, T, 7))
                    for (a, b, last_i) in chunks:
                        nc.tensor.matmul(
                            ps_pv[:, a:b],
                            vstore[:, i, h, :],
                            wbuf[:, u, o + (a - t0):o + (b - t0)],
                            start=(i == 0), stop=(i == last_i),
                        )
                tp = tr.tile([1, T], F32, tag="tp")
                nc.vector.tensor_scalar_add(tp, ps_pv[64:65, :],
                                            tb_sb[0:1, h:h + 1])
                nc.vector.reciprocal_approx_fast(tp, tp)
                gb = tr.tile([64, T], F32, tag="gb")
                nc.gpsimd.partition_broadcast(gb, tp, channels=64)
                r0 = 64 * (h % 2)
                nc.vector.scalar_tensor_tensor(
                    out=ctx[r0:r0 + 64, h // 2, :], in0=ps_pv[0:64, :],
                    scalar=vns_sb[:, h:h + 1], in1=gb,
                    op0=ALU.add, op1=ALU.mult,
                )
            prev_wave_silu = wave_silu

            # W_O halves: cts (0,1) after wave 1, cts (2,3) + combine after wave 3
            if wi in (1, 3):
                cts = (0, 1) if wi == 1 else (2, 3)
                for mt in range(8):
                    for th in range(2):
                        sl = slice(512 * th, 512 * (th + 1))
                        ps_o = pa.tile([128, T], F32, tag="scores", bufs=2)
                        for ci, ct in enumerate(cts):
                            nc.tensor.matmul(
                                ps_o[:, 0:512], wo[:, ct, mt, :], ctx[:, ct, sl],
                                start=(ci == 0), stop=(ci == 1),
                            )
                        if wi == 1:
                            nc.vector.tensor_copy(y_acc[:, mt, sl], ps_o[:, 0:512])
                        else:
                            ysb = tr.tile([128, 512], F32, tag="ysb")
                            nc.vector.tensor_tensor(
                                ysb, ps_o[:, 0:512], y_acc[:, mt, sl], op=ALU.add
                            )
                            nc.sync.dma_start(
                                out=YT.ap()[128 * mt:128 * (mt + 1), sl], in_=ysb
                            )

        pa.release()
        pw.release()
        tr.release()
        pk.release()
        pc.release()

    nc.finalize()
    return nc


def _host_inputs(inputs):
    """Build the 8 per-core input maps from full inputs."""
    X = np.asarray(inputs["X"], dtype=np.float32)
    W_Q = np.asarray(inputs["W_Q"], dtype=np.float32)
    b_Q = np.asarray(inputs["b_Q"], dtype=np.float32)
    W_K = np.asarray(inputs["W_K"], dtype=np.float32)
    b_K = np.asarray(inputs["b_K"], dtype=np.float32)
    W_V = np.asarray(inputs["W_V"], dtype=np.float32)
    b_V = np.asarray(inputs["b_V"], dtype=np.float32)
    sink = np.asarray(inputs["sink_scalars"], dtype=np.float32)
    v_nulls = np.asarray(inputs["v_nulls"], dtype=np.float32)
    W_O = np.asarray(inputs["W_O"], dtype=np.float32)

    XT = np.ascontiguousarray(X[0].T)  # [C, T]

    # channel permutation (evens then odds) within each head's 64 channels
    perm64 = np.concatenate([np.arange(0, 64, 2), np.arange(1, 64, 2)])
    perm512 = (np.arange(8)[:, None] * 64 + perm64[None, :]).reshape(-1)

    # RoPE tables, matching reference float32 math
    invf = (1.0 / (10000.0 ** (np.arange(0, DH, 2, dtype=np.float32) / DH))).astype(
        np.float32
    )
    freqs = np.arange(T, dtype=np.float32)[:, None] * invf[None, :]  # [T, 32]
    cos32 = np.cos(freqs).T  # [32, T]
    sin32 = np.sin(freqs).T
    cos128 = np.tile(cos32, (4, 1)).astype(np.float16)
    sin128 = np.concatenate([-sin32, sin32, -sin32, sin32], axis=0).astype(np.float16)

    # swap matrix: out[p] = q[partner(p)]; lhsT[p', p] = 1 iff p' = partner(p)
    pswap = np.zeros((128, 128), dtype=np.float16)
    for p in range(128):
        partner = p + 32 if (p % 64) < 32 else p - 32
        pswap[partner, p] = 1.0

    # key_self selectors: sel[g][p, h] = 1 iff h == 2g + (p >= 64)
    sel = np.zeros((128, 4, 8), dtype=np.float16)
    for g in range(4):
        sel[0:64, g, 2 * g] = 1.0
        sel[64:128, g, 2 * g + 1] = 1.0

    in_maps = []
    for c in range(N_CORES):
        n, half = c // 2, c % 2
        qs = slice(512 * c, 512 * (c + 1))
        ks = slice(512 * half, 512 * (half + 1))
        heads = np.arange(8 * c, 8 * c + 8)
        sinks = sink[heads]  # [8]
        thr = np.tile((S * sinks).astype(np.float32)[None, :], (128, 1))
        tb = (S * (sinks + 1e-6)).astype(np.float32)[None, :]
        vn = v_nulls[n].reshape(N_HEAD, DH)  # base-head x d
        vns = np.zeros((64, 8), dtype=np.float32)
        for h in range(8):
            bh = (8 * half) + h  # base head index within branch
            vns[:, h] = S * sinks[h] * vn[bh]
        in_maps.append(
            {
                "XT": XT.astype(np.float16),
                "WQ": np.ascontiguousarray(W_Q[:, qs][:, perm512]).astype(np.float16),
                "BQ": np.ascontiguousarray(b_Q[qs][perm512])[None, :].astype(
                    np.float16
                ),
                "WK": np.ascontiguousarray(W_K[:, ks][:, perm512]).astype(np.float16),
                "BK": np.ascontiguousarray(b_K[ks][perm512])[None, :].astype(
                    np.float16
                ),
                "WV": np.ascontiguousarray(W_V[:, ks]).astype(np.float16),
                "BV": np.ascontiguousarray(b_V[ks])[None, :].astype(np.float16),
                "WO": np.ascontiguousarray(0.25 * W_O[n, ks, :]),
                "COS": cos128,
                "SIN": sin128,
                "PSW": pswap,
                "SEL": sel,
                "THR": thr,
                "TB": tb,
                "VNS": vns,
                "ONES": np.ones((1, 512), dtype=np.float16),
            }
        )
    return in_maps


def kernel(**inputs) -> np.ndarray:
    from concourse.bass_utils import run_bass_kernel_spmd

    in_maps = _host_inputs(inputs)
    if _NC_CACHE[0] is None:
        _NC_CACHE[0] = _build_nc()
    nc = _NC_CACHE[0]
    trace = bool(os.environ.get("KBENCH_TRACE"))
    res = run_bass_kernel_spmd(
        nc, in_maps, core_ids=list(range(N_CORES)), trace=trace
    )
    LAST_RESULT[0] = res
    if trace and res.exec_time_ns is not None:
        print(f"HW exec time: {res.exec_time_ns} ns")

    W_O_bias = np.asarray(inputs["W_O_bias"], dtype=np.float32)
    y = np.zeros((T, D_MODEL), dtype=np.float32)
    for r in res.results:
        y += r["YT"].T
    y += W_O_bias.mean(axis=0)[None, :]
    return y[None, :, :]


# revision 16
# speedup vs baseline: 1.0715x; 1.0379x over previous
"""Trainium2 Bass kernel for nn_Attention_65609920414302 (sparse multi-branch attention).

Sharding: 64 total heads (4 branches x 16 sub-heads) split as 8 heads per core
(core c = branch c//2, base-head half c%2). Each core computes Q/K/V projections
for its heads, RoPE, causal thresholded-softplus attention, and a partial W_O
matmul; the host sums the 8 partial outputs.

Math rescaling used on device (S = pi/sqrt(3)):
  reference w_sig = w*sigmoid(S*w) with w = softplus(scores*m), thresholded at sink.
  device   W = silu(S*w) = S*w_sig, thresholded at S*sink,
  probs    = W / (sum_s W + S*(sink+1e-6)),  sink term = S*sink / (...).
The S factors cancel exactly. softplus is composed as ln(1 + exp(m*x)) because
this toolchain has no softplus ACT table; exp/ln/silu phases are ordered with
explicit deps so each wave costs exactly 3 ACT table loads.

Pipeline: 4 waves of 1 head-pair each. Per wave: scores (PE, fp16) -> exp (ACT)
-> causal mask (gpsimd) -> ln (ACT) -> silu (ACT) -> threshold (DVE) -> PV (PE)
-> 1/total (DVE approx recip) -> broadcast (gpsimd) -> context normalize (DVE).
W_O runs in two halves (after waves 1 and 3) accumulating through an fp16 SBUF
buffer so most of it overlaps the ACT phases.
"""

import math
import os
import numpy as np

D_MODEL = 1024
N_HEAD = 16
N_BR = 4
DH = 64
H_TOT = 64
T = 1024
S = math.pi / math.sqrt(3.0)
ATTNSCALE = DH ** -0.5
N_CORES = 8
HPC = 8          # heads per core
KT = 8           # C // 128 contraction tiles
L_LIST = [T - 128 * i for i in range(8)]
O_LIST = [sum(L_LIST[:i]) for i in range(8)]
W_COLS = sum(L_LIST)  # 4608

_NC_CACHE = [None]
LAST_RESULT = [None]  # stash for test harness (exec_time_ns etc.)


def _build_nc():
    import concourse.bass as bass
    from concourse import bacc
    import concourse.mybir as mybir
    import concourse.tile as tile
    from concourse.tile import add_dep_helper
    from concourse.masks import make_identity

    F32 = mybir.dt.float32
    F32R = mybir.dt.float32r
    F16 = mybir.dt.float16
    AF = mybir.ActivationFunctionType
    ALU = mybir.AluOpType

    nc = bacc.Bacc(None, target_bir_lowering=False, debug=False)

    # ---- DRAM parameters (per-core data; same program on all cores) ----
    XT = nc.declare_dram_parameter("XT", [D_MODEL, T], F16, isOutput=False)
    WQ = nc.declare_dram_parameter("WQ", [D_MODEL, 512], F16, isOutput=False)
    BQ = nc.declare_dram_parameter("BQ", [1, 512], F16, isOutput=False)
    WK = nc.declare_dram_parameter("WK", [D_MODEL, 512], F16, isOutput=False)
    BK = nc.declare_dram_parameter("BK", [1, 512], F16, isOutput=False)
    WV = nc.declare_dram_parameter("WV", [D_MODEL, 512], F16, isOutput=False)
    BV = nc.declare_dram_parameter("BV", [1, 512], F16, isOutput=False)
    WO = nc.declare_dram_parameter("WO", [512, D_MODEL], F32R, isOutput=False)
    COS = nc.declare_dram_parameter("COS", [128, T], F16, isOutput=False)
    SIN = nc.declare_dram_parameter("SIN", [128, T], F16, isOutput=False)
    PSW = nc.declare_dram_parameter("PSW", [128, 128], F16, isOutput=False)
    SEL = nc.declare_dram_parameter("SEL", [128, 4, 8], F16, isOutput=False)
    THR = nc.declare_dram_parameter("THR", [128, 8], F32, isOutput=False)
    TB = nc.declare_dram_parameter("TB", [1, 8], F32, isOutput=False)
    VNS = nc.declare_dram_parameter("VNS", [64, 8], F32, isOutput=False)
    ONES = nc.declare_dram_parameter("ONES", [1, 512], F16, isOutput=False)
    YT = nc.declare_dram_parameter("YT", [D_MODEL, T], F32, isOutput=True)

    with tile.TileContext(nc) as tc:
        pc = tc.alloc_tile_pool(name="const", bufs=1)
        pk = tc.alloc_tile_pool(name="keep", bufs=1)
        tr = tc.alloc_tile_pool(name="trans", bufs=2)
        pw = tc.alloc_tile_pool(name="wbuf", bufs=1)
        pp2 = tc.alloc_tile_pool(name="projxv", bufs=1)
        pp1 = tc.alloc_tile_pool(name="projqk", bufs=1)
        pj = tc.alloc_tile_pool(name="psproj", bufs=1, space="PSUM")

        # ---- constants ----
        cos_sb = pc.tile([128, T], F16)
        sin_sb = pc.tile([128, T], F16)
        psw_sb = pc.tile([128, 128], F16)
        sel_sb = pc.tile([128, 4, 8], F16)
        thr_sb = pc.tile([128, 8], F32)
        tb_sb = pc.tile([1, 8], F32)
        vns_sb = pc.tile([64, 8], F32)
        ident = pc.tile([128, 128], F32)
        ones_r = pc.tile([1, 512], F16)
        m_colsb = pc.tile([128, 8, 8], F32)
        m_all = pc.tile([8, T], F32)

        nc.sync.dma_start(out=cos_sb, in_=COS.ap())
        nc.sync.dma_start(out=sin_sb, in_=SIN.ap())
        nc.sync.dma_start(out=psw_sb, in_=PSW.ap())
        nc.sync.dma_start(out=sel_sb, in_=SEL.ap())
        nc.sync.dma_start(out=thr_sb, in_=THR.ap())
        nc.sync.dma_start(out=tb_sb, in_=TB.ap())
        nc.sync.dma_start(out=vns_sb, in_=VNS.ap())
        make_identity(nc, ident)
        nc.sync.dma_start(out=ones_r, in_=ONES.ap())

        # ---- weights ----
        xt = pp2.tile([128, KT, T], F16)
        wv = pp2.tile([128, KT, 512], F16)
        bv = pp2.tile([1, 512], F16)
        wq = pp1.tile([128, KT, 4, 128], F16)
        wk = pp1.tile([128, KT, 4, 128], F16)
        bq = pp1.tile([1, 512], F16)
        bk = pp1.tile([1, 512], F16)
        xt_src = XT.ap().rearrange("(kt p) t -> p kt t", p=128)
        wk_src = WK.ap().rearrange("(kt p) (mt m) -> p kt mt m", p=128, m=128)
        for kt in range(KT):
            nc.sync.dma_start(out=xt[:, kt, :], in_=xt_src[:, kt, :])
            nc.sync.dma_start(out=wk[:, kt, :, :], in_=wk_src[:, kt, :, :])
        nc.sync.dma_start(
            out=wq, in_=WQ.ap().rearrange("(kt p) (mt m) -> p kt mt m", p=128, m=128)
        )
        nc.sync.dma_start(out=wv, in_=WV.ap().rearrange("(kt p) v -> p kt v", p=128))
        nc.sync.dma_start(out=bq, in_=BQ.ap())
        nc.sync.dma_start(out=bk, in_=BK.ap())
        nc.sync.dma_start(out=bv, in_=BV.ap())

        wo = pk.tile([128, 4, 8, 128], F32R)
        nc.sync.dma_start(
            out=wo, in_=WO.ap().rearrange("(ct p) (mt m) -> p ct mt m", p=128, m=128)
        )

        qrope = pk.tile([128, 4, T], F16)
        krope = pk.tile([128, 4, T], F16)
        vstore = pk.tile([128, 8, HPC, 65], F16)
        ctx = pk.tile([128, 4, T], F32R)
        y_acc = pk.tile([128, 8, T], F16)
        nc.vector.memset(vstore[:, :, :, 64:65], 1.0)

        # ---- projection + rope for K and Q ----
        def proj_rope(w_t, b_t, out_t, ks_ps=None):
            for g in range(4):
                ps = pj.tile([128, T], F32, tag="projps", bufs=2)
                for th in range(2):
                    sl = slice(512 * th, 512 * (th + 1))
                    for kt in range(KT):
                        nc.tensor.matmul(
                            ps[:, sl], w_t[:, kt, g, :], xt[:, kt, sl],
                            start=(kt == 0), stop=False,
                        )
                    nc.tensor.matmul(
                        ps[:, sl], b_t[0:1, 128 * g:128 * (g + 1)], ones_r,
                        start=False, stop=True,
                    )
                qsb = tr.tile([128, T], F16, tag="qsb")
                nc.vector.tensor_copy(qsb, ps)
                if ks_ps is not None:
                    # key_self from the pre-RoPE projection (rotation-invariant)
                    k2 = tr.tile([128, T], F16, tag="k2", bufs=1)
                    nc.vector.tensor_tensor(k2, qsb, qsb, op=ALU.mult)
                    for th in range(2):
                        sl = slice(512 * th, 512 * (th + 1))
                        nc.tensor.matmul(
                            ks_ps[:, sl], sel_sb[:, g, :], k2[:, sl],
                            start=(g == 0), stop=(g == 3),
                        )
                sw = pj.tile([128, T], F32, tag="swapps")
                for th in range(2):
                    sl = slice(512 * th, 512 * (th + 1))
                    nc.tensor.matmul(sw[:, sl], psw_sb, qsb[:, sl], start=True,
                                     stop=True)
                t1 = tr.tile([128, T], F16, tag="t1")
                nc.vector.tensor_tensor(t1, qsb, cos_sb, op=ALU.mult)
                t2 = tr.tile([128, T], F16, tag="t2")
                nc.vector.tensor_tensor(t2, sw, sin_sb, op=ALU.mult)
                nc.gpsimd.tensor_tensor(out_t[:, g, :], t1, t2, op=ALU.add)

        ks_ps = pj.tile([8, T], F32, tag="ksps")
        proj_rope(wk, bk, krope, ks_ps=ks_ps)

        # ---- key_self -> m (per-key scale folded into the exp pass) ----
        nc.vector.tensor_scalar_max(m_all, ks_ps, 1e-6)
        nc.vector.reciprocal_approx_fast(m_all, m_all)
        # m = ATTNSCALE / sqrt(key_self) = sqrt(recip / DH)
        nc.scalar.activation(m_all, m_all, AF.Sqrt, scale=1.0 / DH)

        proj_rope(wq, bq, qrope)

        # m column transposes after Q proj so they don't block the PE queue
        for i in range(8):
            mt_ps = pj.tile([128, 8], F32, tag="swapps")
            nc.tensor.transpose(mt_ps, m_all[:, 128 * i:128 * (i + 1)],
                                ident[0:8, 0:8])
            nc.vector.tensor_copy(m_colsb[:, i, :], mt_ps)
        pp1.release()
        pj.release()

        pa = tc.alloc_tile_pool(name="psattn", bufs=1, space="PSUM")

        # ---- attention: 4 waves of one head-pair ----
        prev_wave_silu = []
        for wi in range(4):
            j = wi
            wbuf = pw.tile([128, 2, W_COLS], F16, tag="wbuf", bufs=2)
            exp_insts = []
            for i in range(8):
                t0 = 128 * i
                L = L_LIST[i]
                pss = []
                for u in range(2):
                    h = 2 * j + u
                    g, r0 = h // 2, 64 * (h % 2)
                    ps_s = pa.tile([128, T], F32, tag="scores", bufs=2)
                    for c0 in range(0, L, 512):
                        c1 = min(c0 + 512, L)
                        nc.tensor.matmul(
                            ps_s[:, c0:c1],
                            krope[r0:r0 + 64, g, t0:t0 + 128],
                            qrope[r0:r0 + 64, g, t0 + c0:t0 + c1],
                            start=True, stop=True,
                        )
                    pss.append((h, u, ps_s))
                for h, u, ps_s in pss:
                    o = O_LIST[i]
                    e = nc.scalar.activation(
                        wbuf[:, u, o:o + L], ps_s[:, 0:L], AF.Exp,
                        scale=m_colsb[:, i, h:h + 1],
                    )
                    for si in prev_wave_silu:
                        add_dep_helper(e.ins, si.ins, sync=False,
                                       reason="act table phase order")
                    exp_insts.append(e)
                    # zero the upper-triangular part of the diagonal block
                    nc.gpsimd.affine_select(
                        out=wbuf[:, u, o:o + 128], in_=wbuf[:, u, o:o + 128],
                        compare_op=ALU.is_ge, fill=0.0, base=0,
                        pattern=[[1, 128]], channel_multiplier=-1,
                    )

            if wi == 2:
                # first W_O half, overlapping wave-2's ACT phase
                for mt in range(8):
                    for th in range(2):
                        sl = slice(512 * th, 512 * (th + 1))
                        ps_o = pa.tile([128, 512], F32, tag="wops", bufs=2)
                        for ci, ct in enumerate((0, 1)):
                            nc.tensor.matmul(
                                ps_o, wo[:, ct, mt, :], ctx[:, ct, sl],
                                start=(ci == 0), stop=(ci == 1),
                            )
                        nc.vector.tensor_copy(y_acc[:, mt, sl], ps_o)

            if wi == 0:
                # V projection (t on partitions), overlapping the first exp phase
                for tt_i in range(8):
                    psv = pa.tile([128, T], F32, tag="scores", bufs=2)
                    for kt in range(KT):
                        nc.tensor.matmul(
                            psv[:, 0:512], xt[:, kt, 128 * tt_i:128 * (tt_i + 1)],
                            wv[:, kt, :], start=(kt == 0), stop=False,
                        )
                    nc.tensor.matmul(
                        psv[:, 0:512], ones_r[0:1, 0:128], bv, start=False, stop=True
                    )
                    nc.vector.tensor_copy(
                        vstore[:, tt_i, :, 0:64],
                        psv[:, 0:512].rearrange("p (h d) -> p h d", d=64),
                    )
                pp2.release()

            ln_insts = []
            for u in range(2):
                h = 2 * j + u
                ln = nc.scalar.activation(
                    wbuf[:, u, :], wbuf[:, u, :], AF.Ln, bias=1.0
                )
                for e in exp_insts:
                    add_dep_helper(ln.ins, e.ins, sync=False,
                                   reason="act table phase order")
                ln_insts.append((h, u, ln))
            wave_silu = []
            for h, u, _ln in ln_insts:
                si = nc.scalar.activation(
                    wbuf[:, u, :], wbuf[:, u, :], AF.Silu, scale=S
                )
                for _h2, _u2, l2 in ln_insts:
                    add_dep_helper(si.ins, l2.ins, sync=False,
                                   reason="act table phase order")
                wave_silu.append(si)
                # threshold: w = (w >= thr) * w
                nc.vector.scalar_tensor_tensor(
                    out=wbuf[:, u, :], in0=wbuf[:, u, :],
                    scalar=thr_sb[:, h:h + 1], in1=wbuf[:, u, :],
                    op0=ALU.is_ge, op1=ALU.mult,
                )
                ps_pv = pa.tile([65, T], F32, tag="pv", bufs=1)
                for i in range(8):
                    t0 = 128 * i
                    o = O_LIST[i]
                    chunks = []
                    if t0 < 512:
                        chunks.append((t0, 512, 3))
                        chunks.append((512, T, 7))
                    else:
                        chunks.append((t0, T, 7))
                    for (a, b, last_i) in chunks:
                        nc.tensor.matmul(
                            ps_pv[:, a:b],
                            vstore[:, i, h, :],
                            wbuf[:, u, o + (a - t0):o + (b - t0)],
                            start=(i == 0), stop=(i == last_i),
                        )
                tp = tr.tile([1, T], F32, tag="tp")
                nc.vector.tensor_scalar_add(tp, ps_pv[64:65, :],
                                            tb_sb[0:1, h:h + 1])
                nc.vector.reciprocal_approx_fast(tp, tp)
                gb = tr.tile([64, T], F32, tag="gb")
                nc.gpsimd.partition_broadcast(gb, tp, channels=64)
                r0 = 64 * (h % 2)
                nc.vector.scalar_tensor_tensor(
                    out=ctx[r0:r0 + 64, h // 2, :], in0=ps_pv[0:64, :],
                    scalar=vns_sb[:, h:h + 1], in1=gb,
                    op0=ALU.add, op1=ALU.mult,
                )
            prev_wave_silu = wave_silu

            # second W_O half + combine + writeback, after the last wave
            if wi == 3:
                for mt in range(8):
                    for th in range(2):
                        sl = slice(512 * th, 512 * (th + 1))
                        ps_o = pa.tile([128, 512], F32, tag="wops", bufs=2)
                        for ci, ct in enumerate((2, 3)):
                            nc.tensor.matmul(
                                ps_o, wo[:, ct, mt, :], ctx[:, ct, sl],
                                start=(ci == 0), stop=(ci == 1),
                            )
                        ysb = tr.tile([128, 512], F32, tag="ysb")
                        nc.vector.tensor_tensor(
                            ysb, ps_o, y_acc[:, mt, sl], op=ALU.add
                        )
                        nc.sync.dma_start(
                            out=YT.ap()[128 * mt:128 * (mt + 1), sl], in_=ysb
                        )

        pa.release()
        pw.release()
        tr.release()
        pk.release()
        pc.release()

    nc.finalize()
    return nc


def _host_inputs(inputs):
    """Build the 8 per-core input maps from full inputs."""
    X = np.asarray(inputs["X"], dtype=np.float32)
    W_Q = np.asarray(inputs["W_Q"], dtype=np.float32)
    b_Q = np.asarray(inputs["b_Q"], dtype=np.float32)
    W_K = np.asarray(inputs["W_K"], dtype=np.float32)
    b_K = np.asarray(inputs["b_K"], dtype=np.float32)
    W_V = np.asarray(inputs["W_V"], dtype=np.float32)
    b_V = np.asarray(inputs["b_V"], dtype=np.float32)
    sink = np.asarray(inputs["sink_scalars"], dtype=np.float32)
    v_nulls = np.asarray(inputs["v_nulls"], dtype=np.float32)
    W_O = np.asarray(inputs["W_O"], dtype=np.float32)

    XT = np.ascontiguousarray(X[0].T)  # [C, T]

    # channel permutation (evens then odds) within each head's 64 channels
    perm64 = np.concatenate([np.arange(0, 64, 2), np.arange(1, 64, 2)])
    perm512 = (np.arange(8)[:, None] * 64 + perm64[None, :]).reshape(-1)

    # RoPE tables, matching reference float32 math
    invf = (1.0 / (10000.0 ** (np.arange(0, DH, 2, dtype=np.float32) / DH))).astype(
        np.float32
    )
    freqs = np.arange(T, dtype=np.float32)[:, None] * invf[None, :]  # [T, 32]
    cos32 = np.cos(freqs).T  # [32, T]
    sin32 = np.sin(freqs).T
    cos128 = np.tile(cos32, (4, 1)).astype(np.float16)
    sin128 = np.concatenate([-sin32, sin32, -sin32, sin32], axis=0).astype(np.float16)

    # swap matrix: out[p] = q[partner(p)]; lhsT[p', p] = 1 iff p' = partner(p)
    pswap = np.zeros((128, 128), dtype=np.float16)
    for p in range(128):
        partner = p + 32 if (p % 64) < 32 else p - 32
        pswap[partner, p] = 1.0

    # key_self selectors: sel[g][p, h] = 1 iff h == 2g + (p >= 64)
    sel = np.zeros((128, 4, 8), dtype=np.float16)
    for g in range(4):
        sel[0:64, g, 2 * g] = 1.0
        sel[64:128, g, 2 * g + 1] = 1.0

    in_maps = []
    for c in range(N_CORES):
        n, half = c // 2, c % 2
        qs = slice(512 * c, 512 * (c + 1))
        ks = slice(512 * half, 512 * (half + 1))
        heads = np.arange(8 * c, 8 * c + 8)
        sinks = sink[heads]  # [8]
        thr = np.tile((S * sinks).astype(np.float32)[None, :], (128, 1))
        tb = (S * (sinks + 1e-6)).astype(np.float32)[None, :]
        vn = v_nulls[n].reshape(N_HEAD, DH)  # base-head x d
        vns = np.zeros((64, 8), dtype=np.float32)
        for h in range(8):
            bh = (8 * half) + h  # base head index within branch
            vns[:, h] = S * sinks[h] * vn[bh]
        in_maps.append(
            {
                "XT": XT.astype(np.float16),
                "WQ": np.ascontiguousarray(W_Q[:, qs][:, perm512]).astype(np.float16),
                "BQ": np.ascontiguousarray(b_Q[qs][perm512])[None, :].astype(
                    np.float16
                ),
                "WK": np.ascontiguousarray(W_K[:, ks][:, perm512]).astype(np.float16),
                "BK": np.ascontiguousarray(b_K[ks][perm512])[None, :].astype(
                    np.float16
                ),
                "WV": np.ascontiguousarray(W_V[:, ks]).astype(np.float16),
                "BV": np.ascontiguousarray(b_V[ks])[None, :].astype(np.float16),
                "WO": np.ascontiguousarray(0.25 * W_O[n, ks, :]),
                "COS": cos128,
                "SIN": sin128,
                "PSW": pswap,
                "SEL": sel,
                "THR": thr,
                "TB": tb,
                "VNS": vns,
                "ONES": np.ones((1, 512), dtype=np.float16),
            }
        )
    return in_maps


def kernel(**inputs) -> np.ndarray:
    from concourse.bass_utils import run_bass_kernel_spmd

    in_maps = _host_inputs(inputs)
    if _NC_CACHE[0] is None:
        _NC_CACHE[0] = _build_nc()
    nc = _NC_CACHE[0]
    trace = bool(os.environ.get("KBENCH_TRACE"))
    res = run_bass_kernel_spmd(
        nc, in_maps, core_ids=list(range(N_CORES)), trace=trace
    )
    LAST_RESULT[0] = res
    if trace and res.exec_time_ns is not None:
        print(f"HW exec time: {res.exec_time_ns} ns")

    W_O_bias = np.asarray(inputs["W_O_bias"], dtype=np.float32)
    y = np.zeros((T, D_MODEL), dtype=np.float32)
    for r in res.results:
        y += r["YT"].T
    y += W_O_bias.mean(axis=0)[None, :]
    return y[None, :, :]


# revision 19
# speedup vs baseline: 1.1180x; 1.0434x over previous
"""Trainium2 Bass kernel for nn_Attention_65609920414302 (sparse multi-branch attention).

Sharding: 64 total heads (4 branches x 16 sub-heads) split as 8 heads per core
(core c = branch c//2, base-head half c%2). Each core computes Q/K/V projections
for its heads, RoPE, causal thresholded-softplus attention, and a partial W_O
matmul; the host sums the 8 partial outputs.

Math rescaling used on device (S = pi/sqrt(3)):
  reference w_sig = w*sigmoid(S*w) with w = softplus(scores*m), thresholded at sink.
  device   W = silu(S*w) = S*w_sig, thresholded at S*sink,
  probs    = W / (sum_s W + S*(sink+1e-6)),  sink term = S*sink / (...).
The S factors cancel exactly. softplus is composed as ln(1 + exp(m*x)) because
this toolchain has no softplus ACT table; exp/ln/silu phases are ordered with
explicit deps so each wave costs exactly 3 ACT table loads.

Pipeline: 4 waves of 1 head-pair each. Per wave: scores (PE, fp16) -> exp (ACT)
-> causal mask (gpsimd) -> ln (ACT) -> silu (ACT) -> threshold (DVE) -> PV (PE)
-> 1/total (DVE approx recip) -> broadcast (gpsimd) -> context normalize (DVE).
W_O runs in two halves (after waves 1 and 3) accumulating through an fp16 SBUF
buffer so most of it overlaps the ACT phases.
"""

import math
import os
import numpy as np

D_MODEL = 1024
N_HEAD = 16
N_BR = 4
DH = 64
H_TOT = 64
T = 1024
S = math.pi / math.sqrt(3.0)
ATTNSCALE = DH ** -0.5
N_CORES = 8
HPC = 8          # heads per core
KT = 8           # C // 128 contraction tiles
L_LIST = [T - 128 * i for i in range(8)]
O_LIST = [sum(L_LIST[:i]) for i in range(8)]
W_COLS = sum(L_LIST)  # 4608

_NC_CACHE = [None]
LAST_RESULT = [None]  # stash for test harness (exec_time_ns etc.)


def _build_nc():
    import concourse.bass as bass
    from concourse import bacc
    import concourse.mybir as mybir
    import concourse.tile as tile
    from concourse.tile import add_dep_helper
    from concourse.masks import make_identity

    F32 = mybir.dt.float32
    F32R = mybir.dt.float32r
    F16 = mybir.dt.float16
    AF = mybir.ActivationFunctionType
    ALU = mybir.AluOpType

    nc = bacc.Bacc(None, target_bir_lowering=False, debug=False)

    # ---- DRAM parameters (per-core data; same program on all cores) ----
    XT = nc.declare_dram_parameter("XT", [D_MODEL, T], F16, isOutput=False)
    WQ = nc.declare_dram_parameter("WQ", [D_MODEL, 512], F16, isOutput=False)
    BQ = nc.declare_dram_parameter("BQ", [1, 512], F16, isOutput=False)
    WK = nc.declare_dram_parameter("WK", [D_MODEL, 512], F16, isOutput=False)
    BK = nc.declare_dram_parameter("BK", [1, 512], F16, isOutput=False)
    WV = nc.declare_dram_parameter("WV", [D_MODEL, 512], F16, isOutput=False)
    BV = nc.declare_dram_parameter("BV", [1, 512], F16, isOutput=False)
    WO = nc.declare_dram_parameter("WO", [512, D_MODEL], F32R, isOutput=False)
    COS = nc.declare_dram_parameter("COS", [128, T], F16, isOutput=False)
    SIN = nc.declare_dram_parameter("SIN", [128, T], F16, isOutput=False)
    PSW = nc.declare_dram_parameter("PSW", [128, 128], F16, isOutput=False)
    SEL = nc.declare_dram_parameter("SEL", [128, 4, 8], F16, isOutput=False)
    THR = nc.declare_dram_parameter("THR", [128, 8], F32, isOutput=False)
    TB = nc.declare_dram_parameter("TB", [1, 8], F32, isOutput=False)
    VNS = nc.declare_dram_parameter("VNS", [64, 8], F32, isOutput=False)
    ONES = nc.declare_dram_parameter("ONES", [1, 512], F16, isOutput=False)
    YT = nc.declare_dram_parameter("YT", [D_MODEL, T], F32, isOutput=True)

    with tile.TileContext(nc) as tc:
        pc = tc.alloc_tile_pool(name="const", bufs=1)
        pk = tc.alloc_tile_pool(name="keep", bufs=1)
        tr = tc.alloc_tile_pool(name="trans", bufs=2)
        pw = tc.alloc_tile_pool(name="wbuf", bufs=1)
        pp2 = tc.alloc_tile_pool(name="projxv", bufs=1)
        pp1 = tc.alloc_tile_pool(name="projqk", bufs=1)
        pj = tc.alloc_tile_pool(name="psproj", bufs=1, space="PSUM")

        # ---- constants ----
        cos_sb = pc.tile([128, T], F16)
        sin_sb = pc.tile([128, T], F16)
        psw_sb = pc.tile([128, 128], F16)
        sel_sb = pc.tile([128, 4, 8], F16)
        thr_sb = pc.tile([128, 8], F32)
        tb_sb = pc.tile([1, 8], F32)
        vns_sb = pc.tile([64, 8], F32)
        ident = pc.tile([128, 128], F32)
        ones_r = pc.tile([1, 512], F16)
        m_colsb = pc.tile([128, 8, 8], F32)
        m_all = pc.tile([8, T], F32)

        nc.sync.dma_start(out=cos_sb, in_=COS.ap())
        nc.sync.dma_start(out=sin_sb, in_=SIN.ap())
        nc.sync.dma_start(out=psw_sb, in_=PSW.ap())
        nc.sync.dma_start(out=sel_sb, in_=SEL.ap())
        nc.sync.dma_start(out=thr_sb, in_=THR.ap())
        nc.sync.dma_start(out=tb_sb, in_=TB.ap())
        nc.sync.dma_start(out=vns_sb, in_=VNS.ap())
        make_identity(nc, ident)
        nc.sync.dma_start(out=ones_r, in_=ONES.ap())

        # ---- weights ----
        xt = pp2.tile([128, KT, T], F16)
        wv = pp2.tile([128, KT, 512], F16)
        bv = pp2.tile([1, 512], F16)
        wq = pp1.tile([128, KT, 4, 128], F16)
        wk = pp1.tile([128, KT, 4, 128], F16)
        bq = pp1.tile([1, 512], F16)
        bk = pp1.tile([1, 512], F16)
        xt_src = XT.ap().rearrange("(kt p) t -> p kt t", p=128)
        wk_src = WK.ap().rearrange("(kt p) (mt m) -> p kt mt m", p=128, m=128)
        for kt in range(KT):
            nc.sync.dma_start(out=xt[:, kt, :], in_=xt_src[:, kt, :])
            nc.sync.dma_start(out=wk[:, kt, :, :], in_=wk_src[:, kt, :, :])
        nc.sync.dma_start(
            out=wq, in_=WQ.ap().rearrange("(kt p) (mt m) -> p kt mt m", p=128, m=128)
        )
        nc.sync.dma_start(out=wv, in_=WV.ap().rearrange("(kt p) v -> p kt v", p=128))
        nc.sync.dma_start(out=bq, in_=BQ.ap())
        nc.sync.dma_start(out=bk, in_=BK.ap())
        nc.sync.dma_start(out=bv, in_=BV.ap())

        wo = pk.tile([128, 4, 8, 128], F32R)
        nc.sync.dma_start(
            out=wo, in_=WO.ap().rearrange("(ct p) (mt m) -> p ct mt m", p=128, m=128)
        )

        qrope = pk.tile([128, 4, T], F16)
        krope = pk.tile([128, 4, T], F16)
        vstore = pk.tile([128, 8, HPC, 65], F16)
        ctx = pk.tile([128, 4, T], F32R)
        y_acc = pk.tile([128, 8, T], F16)
        nc.vector.memset(vstore[:, :, :, 64:65], 1.0)

        # ---- projection + rope for K and Q ----
        def proj_rope(w_t, b_t, out_t, ks_ps=None):
            for g in range(4):
                ps = pj.tile([128, T], F32, tag="projps", bufs=2)
                for th in range(2):
                    sl = slice(512 * th, 512 * (th + 1))
                    for kt in range(KT):
                        nc.tensor.matmul(
                            ps[:, sl], w_t[:, kt, g, :], xt[:, kt, sl],
                            start=(kt == 0), stop=False,
                        )
                    nc.tensor.matmul(
                        ps[:, sl], b_t[0:1, 128 * g:128 * (g + 1)], ones_r,
                        start=False, stop=True,
                    )
                qsb = tr.tile([128, T], F16, tag="qsb")
                nc.vector.tensor_copy(qsb, ps)
                if ks_ps is not None:
                    # key_self from the pre-RoPE projection (rotation-invariant)
                    k2 = tr.tile([128, T], F16, tag="k2", bufs=1)
                    nc.vector.tensor_tensor(k2, qsb, qsb, op=ALU.mult)
                    for th in range(2):
                        sl = slice(512 * th, 512 * (th + 1))
                        nc.tensor.matmul(
                            ks_ps[:, sl], sel_sb[:, g, :], k2[:, sl],
                            start=(g == 0), stop=(g == 3),
                        )
                sw = pj.tile([128, T], F32, tag="swapps")
                for th in range(2):
                    sl = slice(512 * th, 512 * (th + 1))
                    nc.tensor.matmul(sw[:, sl], psw_sb, qsb[:, sl], start=True,
                                     stop=True)
                t1 = tr.tile([128, T], F16, tag="t1")
                nc.vector.tensor_tensor(t1, qsb, cos_sb, op=ALU.mult)
                t2 = tr.tile([128, T], F16, tag="t2")
                nc.vector.tensor_tensor(t2, sw, sin_sb, op=ALU.mult)
                nc.gpsimd.tensor_tensor(out_t[:, g, :], t1, t2, op=ALU.add)

        # warm up the PE clock (HAM) with dummy matmuls while DMAs stream in
        wu_ps = pj.tile([1, 512], F32, tag="swapps")
        for _ in range(24):
            nc.tensor.matmul(wu_ps, ones_r[0:1, 0:1], ones_r, start=True,
                             stop=True)

        ks_ps = pj.tile([8, T], F32, tag="ksps")
        proj_rope(wk, bk, krope, ks_ps=ks_ps)

        # ---- key_self -> m (per-key scale folded into the exp pass) ----
        nc.vector.tensor_scalar_max(m_all, ks_ps, 1e-6)
        nc.vector.reciprocal_approx_fast(m_all, m_all)
        # m = ATTNSCALE / sqrt(key_self) = sqrt(recip / DH)
        nc.scalar.activation(m_all, m_all, AF.Sqrt, scale=1.0 / DH)

        proj_rope(wq, bq, qrope)

        # m column transposes after Q proj so they don't block the PE queue
        for i in range(8):
            mt_ps = pj.tile([128, 8], F32, tag="swapps")
            nc.tensor.transpose(mt_ps, m_all[:, 128 * i:128 * (i + 1)],
                                ident[0:8, 0:8])
            nc.vector.tensor_copy(m_colsb[:, i, :], mt_ps)
        pp1.release()
        pj.release()

        pa = tc.alloc_tile_pool(name="psattn", bufs=1, space="PSUM")

        # ---- attention: 4 waves of one head-pair ----
        prev_wave_silu = []
        for wi in range(4):
            j = wi
            wbuf = pw.tile([128, 2, W_COLS], F16, tag="wbuf", bufs=2)
            exp_insts = []
            for i in range(8):
                t0 = 128 * i
                L = L_LIST[i]
                pss = []
                for u in range(2):
                    h = 2 * j + u
                    g, r0 = h // 2, 64 * (h % 2)
                    ps_s = pa.tile([128, T], F32, tag="scores", bufs=2)
                    for c0 in range(0, L, 512):
                        c1 = min(c0 + 512, L)
                        nc.tensor.matmul(
                            ps_s[:, c0:c1],
                            krope[r0:r0 + 64, g, t0:t0 + 128],
                            qrope[r0:r0 + 64, g, t0 + c0:t0 + c1],
                            start=True, stop=True,
                        )
                    pss.append((h, u, ps_s))
                for h, u, ps_s in pss:
                    o = O_LIST[i]
                    e = nc.scalar.activation(
                        wbuf[:, u, o:o + L], ps_s[:, 0:L], AF.Exp,
                        scale=m_colsb[:, i, h:h + 1],
                    )
                    for si in prev_wave_silu:
                        add_dep_helper(e.ins, si.ins, sync=False,
                                       reason="act table phase order")
                    exp_insts.append(e)
                    # zero the upper-triangular part of the diagonal block
                    nc.gpsimd.affine_select(
                        out=wbuf[:, u, o:o + 128], in_=wbuf[:, u, o:o + 128],
                        compare_op=ALU.is_ge, fill=0.0, base=0,
                        pattern=[[1, 128]], channel_multiplier=-1,
                    )

            if wi == 2:
                # first W_O half, overlapping wave-2's ACT phase
                for mt in range(8):
                    for th in range(2):
                        sl = slice(512 * th, 512 * (th + 1))
                        ps_o = pa.tile([128, 512], F32, tag="pv", bufs=2)
                        for ci, ct in enumerate((0, 1)):
                            nc.tensor.matmul(
                                ps_o, wo[:, ct, mt, :], ctx[:, ct, sl],
                                start=(ci == 0), stop=(ci == 1),
                            )
                        nc.vector.tensor_copy(y_acc[:, mt, sl], ps_o)

            if wi == 0:
                # V projection (t on partitions), overlapping the first exp phase
                for tt_i in range(8):
                    psv = pa.tile([128, T], F32, tag="scores", bufs=2)
                    for kt in range(KT):
                        nc.tensor.matmul(
                            psv[:, 0:512], xt[:, kt, 128 * tt_i:128 * (tt_i + 1)],
                            wv[:, kt, :], start=(kt == 0), stop=False,
                        )
                    nc.tensor.matmul(
                        psv[:, 0:512], ones_r[0:1, 0:128], bv, start=False, stop=True
                    )
                    nc.vector.tensor_copy(
                        vstore[:, tt_i, :, 0:64],
                        psv[:, 0:512].rearrange("p (h d) -> p h d", d=64),
                    )
                pp2.release()

            ln_insts = []
            for u in range(2):
                h = 2 * j + u
                ln = nc.scalar.activation(
                    wbuf[:, u, :], wbuf[:, u, :], AF.Ln, bias=1.0
                )
                for e in exp_insts:
                    add_dep_helper(ln.ins, e.ins, sync=False,
                                   reason="act table phase order")
                ln_insts.append((h, u, ln))
            wave_silu = []
            for h, u, _ln in ln_insts:
                si = nc.scalar.activation(
                    wbuf[:, u, :], wbuf[:, u, :], AF.Silu, scale=S
                )
                for _h2, _u2, l2 in ln_insts:
                    add_dep_helper(si.ins, l2.ins, sync=False,
                                   reason="act table phase order")
                wave_silu.append(si)
                # threshold: w = (w >= thr) * w
                nc.vector.scalar_tensor_tensor(
                    out=wbuf[:, u, :], in0=wbuf[:, u, :],
                    scalar=thr_sb[:, h:h + 1], in1=wbuf[:, u, :],
                    op0=ALU.is_ge, op1=ALU.mult,
                )
                ps_pv = pa.tile([65, T], F32, tag="pv", bufs=2)
                for i in range(8):
                    t0 = 128 * i
                    o = O_LIST[i]
                    chunks = []
                    if t0 < 512:
                        chunks.append((t0, 512, 3))
                        chunks.append((512, T, 7))
                    else:
                        chunks.append((t0, T, 7))
                    for (a, b, last_i) in chunks:
                        nc.tensor.matmul(
                            ps_pv[:, a:b],
                            vstore[:, i, h, :],
                            wbuf[:, u, o + (a - t0):o + (b - t0)],
                            start=(i == 0), stop=(i == last_i),
                        )
                tp = tr.tile([1, T], F32, tag="tp")
                nc.vector.tensor_scalar_add(tp, ps_pv[64:65, :],
                                            tb_sb[0:1, h:h + 1])
                nc.vector.reciprocal_approx_fast(tp, tp)
                gb = tr.tile([64, T], F32, tag="gb")
                nc.gpsimd.partition_broadcast(gb, tp, channels=64)
                r0 = 64 * (h % 2)
                nc.vector.scalar_tensor_tensor(
                    out=ctx[r0:r0 + 64, h // 2, :], in0=ps_pv[0:64, :],
                    scalar=vns_sb[:, h:h + 1], in1=gb,
                    op0=ALU.add, op1=ALU.mult,
                )
            prev_wave_silu = wave_silu

            # second W_O half + combine + writeback, after the last wave
            if wi == 3:
                for mt in range(8):
                    for th in range(2):
                        sl = slice(512 * th, 512 * (th + 1))
                        ps_o = pa.tile([128, 512], F32, tag="pv", bufs=2)
                        for ci, ct in enumerate((2, 3)):
                            nc.tensor.matmul(
                                ps_o, wo[:, ct, mt, :], ctx[:, ct, sl],
                                start=(ci == 0), stop=(ci == 1),
                            )
                        ysb = tr.tile([128, 512], F32, tag="ysb")
                        nc.vector.tensor_tensor(
                            ysb, ps_o, y_acc[:, mt, sl], op=ALU.add
                        )
                        nc.sync.dma_start(
                            out=YT.ap()[128 * mt:128 * (mt + 1), sl], in_=ysb
                        )

        pa.release()
        pw.release()
        tr.release()
        pk.release()
        pc.release()

    nc.finalize()
    return nc


def _host_inputs(inputs):
    """Build the 8 per-core input maps from full inputs."""
    X = np.asarray(inputs["X"], dtype=np.float32)
    W_Q = np.asarray(inputs["W_Q"], dtype=np.float32)
    b_Q = np.asarray(inputs["b_Q"], dtype=np.float32)
    W_K = np.asarray(inputs["W_K"], dtype=np.float32)
    b_K = np.asarray(inputs["b_K"], dtype=np.float32)
    W_V = np.asarray(inputs["W_V"], dtype=np.float32)
    b_V = np.asarray(inputs["b_V"], dtype=np.float32)
    sink = np.asarray(inputs["sink_scalars"], dtype=np.float32)
    v_nulls = np.asarray(inputs["v_nulls"], dtype=np.float32)
    W_O = np.asarray(inputs["W_O"], dtype=np.float32)

    XT = np.ascontiguousarray(X[0].T)  # [C, T]

    # channel permutation (evens then odds) within each head's 64 channels
    perm64 = np.concatenate([np.arange(0, 64, 2), np.arange(1, 64, 2)])
    perm512 = (np.arange(8)[:, None] * 64 + perm64[None, :]).reshape(-1)

    # RoPE tables, matching reference float32 math
    invf = (1.0 / (10000.0 ** (np.arange(0, DH, 2, dtype=np.float32) / DH))).astype(
        np.float32
    )
    freqs = np.arange(T, dtype=np.float32)[:, None] * invf[None, :]  # [T, 32]
    cos32 = np.cos(freqs).T  # [32, T]
    sin32 = np.sin(freqs).T
    cos128 = np.tile(cos32, (4, 1)).astype(np.float16)
    sin128 = np.concatenate([-sin32, sin32, -sin32, sin32], axis=0).astype(np.float16)

    # swap matrix: out[p] = q[partner(p)]; lhsT[p', p] = 1 iff p' = partner(p)
    pswap = np.zeros((128, 128), dtype=np.float16)
    for p in range(128):
        partner = p + 32 if (p % 64) < 32 else p - 32
        pswap[partner, p] = 1.0

    # key_self selectors: sel[g][p, h] = 1 iff h == 2g + (p >= 64)
    sel = np.zeros((128, 4, 8), dtype=np.float16)
    for g in range(4):
        sel[0:64, g, 2 * g] = 1.0
        sel[64:128, g, 2 * g + 1] = 1.0

    in_maps = []
    for c in range(N_CORES):
        n, half = c // 2, c % 2
        qs = slice(512 * c, 512 * (c + 1))
        ks = slice(512 * half, 512 * (half + 1))
        heads = np.arange(8 * c, 8 * c + 8)
        sinks = sink[heads]  # [8]
        thr = np.tile((S * sinks).astype(np.float32)[None, :], (128, 1))
        tb = (S * (sinks + 1e-6)).astype(np.float32)[None, :]
        vn = v_nulls[n].reshape(N_HEAD, DH)  # base-head x d
        vns = np.zeros((64, 8), dtype=np.float32)
        for h in range(8):
            bh = (8 * half) + h  # base head index within branch
            vns[:, h] = S * sinks[h] * vn[bh]
        in_maps.append(
            {
                "XT": XT.astype(np.float16),
                "WQ": np.ascontiguousarray(W_Q[:, qs][:, perm512]).astype(np.float16),
                "BQ": np.ascontiguousarray(b_Q[qs][perm512])[None, :].astype(
                    np.float16
                ),
                "WK": np.ascontiguousarray(W_K[:, ks][:, perm512]).astype(np.float16),
                "BK": np.ascontiguousarray(b_K[ks][perm512])[None, :].astype(
                    np.float16
                ),
                "WV": np.ascontiguousarray(W_V[:, ks]).astype(np.float16),
                "BV": np.ascontiguousarray(b_V[ks])[None, :].astype(np.float16),
                "WO": np.ascontiguousarray(0.25 * W_O[n, ks, :]),
                "COS": cos128,
                "SIN": sin128,
                "PSW": pswap,
                "SEL": sel,
                "THR": thr,
                "TB": tb,
                "VNS": vns,
                "ONES": np.ones((1, 512), dtype=np.float16),
            }
        )
    return in_maps


def kernel(**inputs) -> np.ndarray:
    from concourse.bass_utils import run_bass_kernel_spmd

    in_maps = _host_inputs(inputs)
    if _NC_CACHE[0] is None:
        _NC_CACHE[0] = _build_nc()
    nc = _NC_CACHE[0]
    trace = bool(os.environ.get("KBENCH_TRACE"))
    res = run_bass_kernel_spmd(
        nc, in_maps, core_ids=list(range(N_CORES)), trace=trace
    )
    LAST_RESULT[0] = res
    if trace and res.exec_time_ns is not None:
        print(f"HW exec time: {res.exec_time_ns} ns")

    W_O_bias = np.asarray(inputs["W_O_bias"], dtype=np.float32)
    y = np.zeros((T, D_MODEL), dtype=np.float32)
    for r in res.results:
        y += r["YT"].T
    y += W_O_bias.mean(axis=0)[None, :]
    return y[None, :, :]
